# revision 1
# baseline (speedup 1.0000x reference)
"""Trainium2 Bass kernel for ClimateConditionedGAT (GATConv + one-hot prior gate).

Strategy (8 NeuronCores, SPMD single NEFF):
  - Nodes sharded by destination: core c owns dst nodes [c*6250, (c+1)*6250).
  - Phase 1 (replicated on every core): T[n] = [h(256) | a_src(4) | a_dst(4)]
    where h = x @ W_gat, a_* = per-head dots; stored in a DRAM table with
    1280B rows (dma_gather needs 256B-multiple rows/strides), split into two
    25001-row source buckets (dma_gather indices are int16) each with a
    sentinel row (a_src = -1e30 => exp -> 0) used for padding edges.
    A[n] = a_dst[n] (flat [N,4], gathered as [N/16, 64] rows by dst//16).
  - Phase 2: edges partitioned by (dst-window of 128, src bucket), sorted on
    host; per <=8-block group one dma_gather of T rows by src and one of A
    rows by dst//16; batched DVE/ACT ops compute ex = exp(leaky(asrc+adst));
    per 128-edge block a selection matrix S[e,d] = (dst_local==d) built by
    iota-compare feeds a PE matmul accumulating [sum ex*h | sum ex] into a
    per-window PSUM tile; window epilogue normalizes, averages heads, adds
    bias, computes prior = kg_onehot @ W_prior and gates the two.
All float math on device; host does integer edge partitioning/packing only.
"""
import sys
import numpy as np

if "/opt/trn_rl_repo" not in sys.path:
    sys.path.insert(0, "/opt/trn_rl_repo")

from contextlib import ExitStack

import concourse.bass as bass
import concourse.bacc as bacc
import concourse.mybir as mybir
import concourse.tile as tile
from concourse.bass_utils import run_bass_kernel_spmd

P = 128
N = 50000
DIN = 128
HEADS = 4
F = 64
HF = HEADS * F          # 256
NCORES = 8
SH = N // NCORES        # 6250 dst nodes per core
ROW = 320               # T row in f32 elems (1280 B)
BUCK_N = 25000          # real nodes per source bucket
BROWS = BUCK_N + 1      # bucket rows incl sentinel
TROWS = 2 * BROWS       # 50002
SENT = BUCK_N           # bucket-local sentinel index
WIN = P                 # dst window size
NWIN = (SH + WIN - 1) // WIN    # 49 windows per core
GMAX = 8                # max 128-edge blocks per dma_gather (1024 idx limit)
NEG_SLOPE = 0.2
dt = mybir.dt
Alu = mybir.AluOpType
Act = mybir.ActivationFunctionType


# ----------------------------------------------------------------------------
# host-side edge preparation (integer/layout work only)
# ----------------------------------------------------------------------------

def _pack_idx16(idx):
    """Pack int array -> [128, ceil(n/16)] int16 (wrap 16, replicate x8)."""
    n = len(idx)
    cols = (n + 15) // 16
    a = np.zeros((16, cols), np.int16)
    a[np.arange(n) % 16, np.arange(n) // 16] = idx.astype(np.int16)
    return np.tile(a, (8, 1))


def _prep_edges(edge_index):
    src = np.asarray(edge_index[0], dtype=np.int64)
    dst = np.asarray(edge_index[1], dtype=np.int64)
    loops = np.arange(N, dtype=np.int64)
    src = np.concatenate([src, loops])
    dst = np.concatenate([dst, loops])

    core = dst // SH
    w = (dst % SH) // WIN
    b = (src >= BUCK_N).astype(np.int64)

    # group id per edge: (core, w, b)
    gid = (core * NWIN + w) * 2 + b
    ngroups = NCORES * NWIN * 2
    counts = np.bincount(gid, minlength=ngroups).reshape(NCORES, NWIN, 2)

    # uniform block counts across cores (shared NEFF)
    B_wb = np.ceil(counts.max(axis=0) / P).astype(np.int64)      # [NWIN, 2]

    # chunks of <= GMAX blocks per gather
    chunks = []  # list of (w, b, nblk, blk0) in emission order; blk0 = global block idx
    blk0 = 0
    for wi in range(NWIN):
        for bi in range(2):
            rem = int(B_wb[wi, bi])
            while rem > 0:
                g = min(rem, GMAX)
                chunks.append((wi, bi, g, blk0))
                blk0 += g
                rem -= g
    cblk = blk0                      # total blocks per core
    slots = cblk * P

    order = np.argsort(gid, kind="stable")
    src_s, dst_s, gid_s = src[order], dst[order], gid[order]
    # start offset of each (c,w,b) group inside the sorted edge array
    grp_starts = np.zeros(ngroups + 1, np.int64)
    np.cumsum(np.bincount(gid_s, minlength=ngroups), out=grp_starts[1:])

    # slot offset of each (w,b) group inside the padded per-core layout
    slot_off = np.zeros((NWIN, 2), np.int64)
    off = 0
    for wi in range(NWIN):
        for bi in range(2):
            slot_off[wi, bi] = off
            off += int(B_wb[wi, bi]) * P

    midx = np.full((NCORES, slots), SENT, np.int64)   # bucket-local src row
    tidx = np.zeros((NCORES, slots), np.int64)        # dst // 16 (global)
    dmod = np.zeros((NCORES, slots), np.float32)      # dst % 16
    dstf = np.zeros((NCORES, slots), np.float32)      # dst local to window

    for c in range(NCORES):
        for wi in range(NWIN):
            for bi in range(2):
                g = (c * NWIN + wi) * 2 + bi
                s0, s1 = grp_starts[g], grp_starts[g + 1]
                n = s1 - s0
                if n == 0:
                    continue
                o = slot_off[wi, bi]
                es, ed = src_s[s0:s1], dst_s[s0:s1]
                midx[c, o:o + n] = es - bi * BUCK_N
                tidx[c, o:o + n] = ed // 16
                dmod[c, o:o + n] = (ed % 16).astype(np.float32)
                dstf[c, o:o + n] = (ed - (c * SH + wi * WIN)).astype(np.float32)

    midx_p = np.stack([_pack_idx16(midx[c]) for c in range(NCORES)])
    tidx_p = np.stack([_pack_idx16(tidx[c]) for c in range(NCORES)])
    # column-per-block layout for dmod/dstf: slot e -> [e % 128, e // 128]
    dmod_c = dmod.reshape(NCORES, cblk, P).transpose(0, 2, 1).copy()
    dstf_c = dstf.reshape(NCORES, cblk, P).transpose(0, 2, 1).copy()
    return chunks, B_wb, cblk, midx_p, tidx_p, dmod_c, dstf_c


# ----------------------------------------------------------------------------
# device program
# ----------------------------------------------------------------------------

def _build_nc(chunks, cblk):
    nc = bacc.Bacc("TRN2", target_bir_lowering=False, debug=False)

    t_x = nc.dram_tensor("x", [N, DIN], dt.float32, kind="ExternalInput")
    t_wg = nc.dram_tensor("wg", [DIN, HF], dt.float32, kind="ExternalInput")
    t_asb = nc.dram_tensor("attsrc_b", [P, HF], dt.float32, kind="ExternalInput")
    t_adb = nc.dram_tensor("attdst_b", [P, HF], dt.float32, kind="ExternalInput")
    t_bg = nc.dram_tensor("biasg_b", [P, F], dt.float32, kind="ExternalInput")
    t_bp = nc.dram_tensor("bprior_b", [P, F], dt.float32, kind="ExternalInput")
    t_gate = nc.dram_tensor("gate_b", [P, 1], dt.float32, kind="ExternalInput")
    t_wp = nc.dram_tensor("wp", [32, F], dt.float32, kind="ExternalInput")
    t_kg = nc.dram_tensor("kg", [SH, 32], dt.float32, kind="ExternalInput")
    t_iota = nc.dram_tensor("iota_g", [P, GMAX * P], dt.float32, kind="ExternalInput")
    t_io16 = nc.dram_tensor("iota16_g", [P, GMAX * 16], dt.float32, kind="ExternalInput")
    mcols = (cblk * P) // 16
    t_midx = nc.dram_tensor("midx", [P, mcols], dt.int16, kind="ExternalInput")
    t_tidx = nc.dram_tensor("tidx", [P, mcols], dt.int16, kind="ExternalInput")
    t_dmod = nc.dram_tensor("dmod", [P, cblk], dt.float32, kind="ExternalInput")
    t_dstf = nc.dram_tensor("dstf", [P, cblk], dt.float32, kind="ExternalInput")
    t_out = nc.dram_tensor("out", [SH, F], dt.float32, kind="ExternalOutput")

    t_T = nc.dram_tensor("tbl", [TROWS, ROW], dt.float32)
    t_A = nc.dram_tensor("adst", [N + 48, HEADS], dt.float32)
    a_view = bass.AP(t_A[:, :].tensor, 0, [[64, N // 16], [1, 64]])

    from concourse.masks import make_identity

    with tile.TileContext(nc) as tc:
        with ExitStack() as octx:
            cp = octx.enter_context(tc.tile_pool(name="const", bufs=1))

            ident = cp.tile([P, P], dt.float32)
            make_identity(nc, ident[:])
            iota_sb = cp.tile([P, GMAX * P], dt.float32)
            nc.sync.dma_start(iota_sb[:], t_iota[:])
            io16_sb = cp.tile([P, GMAX * 16], dt.float32)
            nc.sync.dma_start(io16_sb[:], t_io16[:])
            wp_sb = cp.tile([32, F], dt.float32)
            nc.sync.dma_start(wp_sb[:], t_wp[:])
            asb = cp.tile([P, HF], dt.float32)
            nc.sync.dma_start(asb[:], t_asb[:])
            adb = cp.tile([P, HF], dt.float32)
            nc.sync.dma_start(adb[:], t_adb[:])
            bg_sb = cp.tile([P, F], dt.float32)
            nc.sync.dma_start(bg_sb[:], t_bg[:])
            bp_sb = cp.tile([P, F], dt.float32)
            nc.sync.dma_start(bp_sb[:], t_bp[:])
            gate_sb = cp.tile([P, 1], dt.float32)
            nc.sync.dma_start(gate_sb[:], t_gate[:])
            g1m_sb = cp.tile([P, 1], dt.float32)
            nc.vector.tensor_scalar(g1m_sb[:], gate_sb[:], -1.0, 1.0,
                                    op0=Alu.mult, op1=Alu.add)
            midx_sb = cp.tile([P, mcols], dt.int16)
            nc.sync.dma_start(midx_sb[:], t_midx[:])
            tidx_sb = cp.tile([P, mcols], dt.int16)
            nc.sync.dma_start(tidx_sb[:], t_tidx[:])
            dmod_sb = cp.tile([P, cblk], dt.float32)
            nc.sync.dma_start(dmod_sb[:], t_dmod[:])
            dstf_sb = cp.tile([P, cblk], dt.float32)
            nc.sync.dma_start(dstf_sb[:], t_dstf[:])

            # W_ext = [W_gat | wsrc(4) | wdst(4)] where wsrc_j = W[:,j]·att_src[j]
            wext = cp.tile([P, HF + 8], dt.float32)
            nc.sync.dma_start(wext[:, 0:HF], t_wg[:])
            with tc.tile_pool(name="wtmp", bufs=2) as wtp:
                for j in range(HEADS):
                    for k, att in enumerate((asb, adb)):
                        tmp = wtp.tile([P, F], dt.float32, tag="wtmp")
                        nc.vector.tensor_tensor(
                            out=tmp[:], in0=wext[:, j * F:(j + 1) * F],
                            in1=att[:, j * F:(j + 1) * F], op=Alu.mult)
                        nc.vector.tensor_reduce(
                            out=wext[:, HF + 4 * k + j:HF + 4 * k + j + 1],
                            in_=tmp[:], axis=mybir.AxisListType.X, op=Alu.add)

            # ---------------- phase 1: build T and A ----------------
            with tc.tile_pool(name="p1sb", bufs=3) as p1, \
                 tc.tile_pool(name="p1ps", bufs=2, space="PSUM") as pp1, \
                 tc.tile_pool(name="p1ac", bufs=2) as p1a:
                ACH = 32  # blocks of a_dst staged per A-write DMA
                for bkt in range(2):
                    nfull = BUCK_N // P           # 195 full blocks
                    nblk = (BUCK_N + P - 1) // P  # incl partial
                    ach_tile = None
                    ach_base = 0
                    for bi in range(nblk):
                        ns = bkt * BUCK_N + bi * P
                        rs = bkt * BROWS + bi * P
                        nr = min(P, BUCK_N - bi * P)
                        full = nr == P
                        ci = bi % ACH
                        if full and ci == 0:
                            ach_tile = p1a.tile([P, ACH, HEADS], dt.float32,
                                                tag="ach")
                            ach_base = ns
                        xb = p1.tile([P, DIN], dt.float32, tag="xb")
                        if nr < P:
                            nc.gpsimd.memset(xb[:], 0.0)
                        nc.sync.dma_start(xb[:nr, :], t_x[ns:ns + nr, :])
                        xt_ps = pp1.tile([P, P], dt.float32, space="PSUM",
                                         tag="xt")
                        nc.tensor.transpose(out=xt_ps[:], in_=xb[:],
                                            identity=ident[:])
                        xt_sb = p1.tile([P, P], dt.float32, tag="xts")
                        nc.scalar.copy(xt_sb[:], xt_ps[:])
                        h_ps = pp1.tile([P, HF + 8], dt.float32, space="PSUM",
                                        tag="hps")
                        nc.tensor.matmul(h_ps[:], lhsT=xt_sb[:], rhs=wext[:],
                                         start=True, stop=True)
                        row = p1.tile([P, HF + 8], dt.float32, tag="row")
                        nc.scalar.copy(row[:], h_ps[:])
                        nc.sync.dma_start(t_T[rs:rs + nr, 0:HF + 8], row[:nr, :])
                        if full:
                            nc.scalar.copy(ach_tile[:, ci, :],
                                           h_ps[:, HF + 4:HF + 8])
                            if ci == ACH - 1 or bi == nfull - 1:
                                nb = ci + 1
                                dst_ap = t_A[ach_base:ach_base + nb * P, :] \
                                    .rearrange("(b p) c -> p b c", p=P)
                                nc.sync.dma_start(dst_ap, ach_tile[:, 0:nb, :])
                        else:
                            # partial tail block: write its a_dst rows directly
                            nc.sync.dma_start(t_A[ns:ns + nr, :],
                                              row[:nr, HF + 4:HF + 8])
                # sentinel rows
                sent = p1.tile([1, ROW], dt.float32, tag="sent")
                nc.gpsimd.memset(sent[:], 0.0)
                nc.gpsimd.memset(sent[:, HF:HF + 4], -1e30)
                nc.sync.dma_start(t_T[SENT:SENT + 1, :], sent[:])
                nc.sync.dma_start(t_T[BROWS + SENT:BROWS + SENT + 1, :], sent[:])

            # ---------------- phase 2: gather / scatter ----------------
            with tc.tile_pool(name="g1p", bufs=2) as g1p, \
                 tc.tile_pool(name="g2p", bufs=2) as g2p, \
                 tc.tile_pool(name="wk", bufs=3) as wk, \
                 tc.tile_pool(name="sp", bufs=3) as sp, \
                 tc.tile_pool(name="accp", bufs=2, space="PSUM") as accp, \
                 tc.tile_pool(name="prp", bufs=2, space="PSUM") as prp, \
                 tc.tile_pool(name="fin", bufs=3) as fin:

                acc_ps = None
                cur_w = -1
                # chunk index ranges per window for start/stop flags
                win_first = {}
                win_last = {}
                for ci, (wi, bi, g, blk0) in enumerate(chunks):
                    win_first.setdefault(wi, ci)
                    win_last[wi] = ci

                for ci, (wi, bi, g, blk0) in enumerate(chunks):
                    if win_first[wi] == ci:
                        acc_ps = accp.tile([P, HF + 4], dt.float32, space="PSUM",
                                           tag="acc")
                        cur_w = wi
                    nidx = g * P
                    ic0 = (blk0 * P) // 16
                    icn = nidx // 16
                    g1 = g1p.tile([P, GMAX, ROW], dt.float32, tag="g1")
                    nc.gpsimd.dma_gather(
                        g1[:, 0:g, :], t_T[bi * BROWS:(bi + 1) * BROWS, :],
                        midx_sb[:, ic0:ic0 + icn], nidx, nidx, ROW)
                    g2 = g2p.tile([P, GMAX, 64], dt.float32, tag="g2")
                    nc.gpsimd.dma_gather(
                        g2[:, 0:g, :], a_view, tidx_sb[:, ic0:ic0 + icn],
                        nidx, nidx, 64)

                    def v3(t, d1, s1, d2, s2):
                        return bass.AP(t.tensor, t.offset,
                                       [t.ap[0], [s1, d1], [s2, d2]])

                    def v4(t, d1, s1, d2, s2, d3, s3):
                        return bass.AP(t.tensor, t.offset,
                                       [t.ap[0], [s1, d1], [s2, d2], [s3, d3]])

                    g1a = g1[:, 0:g, :]
                    g2a = g2[:, 0:g, :]
                    # mask[e, m] = (dst%16 == m)   [P, g, 16]
                    mask = wk.tile([P, GMAX * 16], dt.float32, tag="mask")
                    nc.vector.tensor_tensor(
                        out=v3(mask[:], g, 16, 16, 1),
                        in0=v3(dmod_sb[:, blk0:blk0 + g], g, 1, 16, 0),
                        in1=v3(io16_sb[:, 0:g * 16], g, 16, 16, 1),
                        op=Alu.is_equal)
                    # adst_sel[e, j] = sum_m mask[e,m] * A_g[e, m*4+j]
                    selt = wk.tile([P, GMAX * 64], dt.float32, tag="selt")
                    nc.vector.tensor_tensor(
                        out=v4(selt[:], g, 64, 16, 4, 4, 1),
                        in0=v4(mask[:], g, 16, 16, 1, 4, 0),
                        in1=v4(g2a, g, 64, 16, 4, 4, 1),
                        op=Alu.mult)
                    adst = wk.tile([P, GMAX * 4], dt.float32, tag="adst")
                    nc.vector.tensor_reduce(
                        out=v3(adst[:], g, 4, 4, 1),
                        in_=v4(selt[:], g, 64, 4, 1, 16, 4),
                        axis=mybir.AxisListType.X, op=Alu.add)
                    # alpha = leaky(asrc + adst); ex = exp(alpha)
                    alpha = wk.tile([P, GMAX * 4], dt.float32, tag="alpha")
                    nc.vector.tensor_tensor(
                        out=v3(alpha[:], g, 4, 4, 1),
                        in0=v3(bass.AP(g1a.tensor, g1a.offset + HF,
                                       [g1a.ap[0]]), g, ROW, 4, 1),
                        in1=v3(adst[:], g, 4, 4, 1), op=Alu.add)
                    nc.vector.scalar_tensor_tensor(
                        out=alpha[:, 0:g * 4], in0=alpha[:, 0:g * 4],
                        scalar=NEG_SLOPE, in1=alpha[:, 0:g * 4],
                        op0=Alu.mult, op1=Alu.max)
                    ex = wk.tile([P, GMAX * 4], dt.float32, tag="ex")
                    nc.scalar.activation(ex[:, 0:g * 4], alpha[:, 0:g * 4], Act.Exp)
                    # S[e, d] = (dstf == d)   [P, g, 128]
                    S = sp.tile([P, GMAX * P], dt.float32, tag="S")
                    nc.vector.tensor_tensor(
                        out=v3(S[:], g, P, P, 1),
                        in0=v3(dstf_sb[:, blk0:blk0 + g], g, 1, P, 0),
                        in1=v3(iota_sb[:, 0:g * P], g, P, P, 1),
                        op=Alu.is_equal)
                    # rhs = [ex*h (256) | ex (4)]   [P, g, 260]
                    rhs = wk.tile([P, GMAX * (HF + 4)], dt.float32, tag="rhs")

                    def hview(t, base_step, hoff, nh, hstep):
                        return bass.AP(t.tensor, t.offset + hoff,
                                       [t.ap[0], [base_step, g], [hstep * F, nh],
                                        [1, F]])

                    def exview(hoff):
                        return bass.AP(ex[:].tensor, ex[:].offset + hoff,
                                       [ex[:].ap[0], [4, g], [1, 2], [0, F]])

                    nc.vector.tensor_tensor(
                        out=hview(rhs[:], HF + 4, 0, 2, 1),
                        in0=hview(g1a, ROW, 0, 2, 1),
                        in1=exview(0), op=Alu.mult)
                    nc.gpsimd.tensor_tensor(
                        out=hview(rhs[:], HF + 4, 2 * F, 2, 1),
                        in0=hview(g1a, ROW, 2 * F, 2, 1),
                        in1=exview(2), op=Alu.mult)
                    nc.scalar.copy(
                        bass.AP(rhs[:].tensor, rhs[:].offset + HF,
                                [rhs[:].ap[0], [HF + 4, g], [1, 4]]),
                        v3(ex[:], g, 4, 4, 1))
                    for k in range(g):
                        nc.tensor.matmul(
                            acc_ps[:],
                            lhsT=S[:, k * P:(k + 1) * P],
                            rhs=rhs[:, k * (HF + 4):(k + 1) * (HF + 4)],
                            start=(win_first[wi] == ci and k == 0),
                            stop=(win_last[wi] == ci and k == g - 1))

                    if win_last[wi] == ci:
                        # ---- window epilogue ----
                        nw = min(WIN, SH - wi * WIN)
                        den = fin.tile([P, HEADS], dt.float32, tag="den")
                        nc.vector.tensor_scalar(
                            den[:], acc_ps[:, HF:HF + 4], 1e-16, float(HEADS),
                            op0=Alu.add, op1=Alu.mult)
                        rec = fin.tile([P, HEADS], dt.float32, tag="rec")
                        nc.vector.reciprocal(rec[:], den[:])
                        prod = fin.tile([P, HF], dt.float32, tag="prod")
                        nc.vector.tensor_tensor(
                            out=v3(prod[:], HEADS, F, F, 1),
                            in0=v3(acc_ps[:, 0:HF], HEADS, F, F, 1),
                            in1=v3(rec[:], HEADS, 1, F, 0), op=Alu.mult)
                        gat = fin.tile([P, F], dt.float32, tag="gat")
                        nc.vector.tensor_reduce(
                            out=gat[:],
                            in_=v3(prod[:], F, 1, HEADS, F),
                            axis=mybir.AxisListType.X, op=Alu.add)
                        # prior
                        kgb = fin.tile([P, 32], dt.float32, tag="kgb")
                        if nw < P:
                            nc.gpsimd.memset(kgb[:], 0.0)
                        n0 = wi * WIN
                        nc.sync.dma_start(kgb[:nw, :], t_kg[n0:n0 + nw, :])
                        kgt_ps = prp.tile([32, P], dt.float32, space="PSUM",
                                          tag="kgt")
                        nc.tensor.transpose(out=kgt_ps[:], in_=kgb[:],
                                            identity=ident[:])
                        kgt_sb = fin.tile([32, P], dt.float32, tag="kgts")
                        nc.scalar.copy(kgt_sb[:], kgt_ps[:])
                        pr_ps = prp.tile([P, F], dt.float32, space="PSUM",
                                         tag="prps")
                        nc.tensor.matmul(pr_ps[:], lhsT=kgt_sb[:],
                                         rhs=wp_sb[:], start=True, stop=True)
                        # combine: (1-g)*(gat+bias) + g*(prior+bprior)
                        t1 = fin.tile([P, F], dt.float32, tag="t1")
                        nc.vector.tensor_tensor(out=t1[:], in0=gat[:],
                                                in1=bg_sb[:], op=Alu.add)
                        nc.vector.tensor_scalar_mul(t1[:], t1[:], g1m_sb[:, 0:1])
                        t2 = fin.tile([P, F], dt.float32, tag="t2")
                        nc.vector.tensor_tensor(out=t2[:], in0=pr_ps[:],
                                                in1=bp_sb[:], op=Alu.add)
                        nc.vector.tensor_scalar_mul(t2[:], t2[:], gate_sb[:, 0:1])
                        nc.vector.tensor_tensor(out=t1[:], in0=t1[:], in1=t2[:],
                                                op=Alu.add)
                        nc.sync.dma_start(t_out[n0:n0 + nw, :], t1[:nw, :])

    nc.compile()
    return nc


def kernel(**inputs):
    x = np.ascontiguousarray(np.asarray(inputs["x"], np.float32))
    edge_index = np.asarray(inputs["edge_index"])
    kg = np.ascontiguousarray(np.asarray(inputs["kg_onehot"], np.float32))
    wg = np.ascontiguousarray(np.asarray(inputs["W_gat"], np.float32))
    att_src = np.asarray(inputs["att_src"], np.float32)
    att_dst = np.asarray(inputs["att_dst"], np.float32)
    bias_gat = np.asarray(inputs["bias_gat"], np.float32)
    wp = np.ascontiguousarray(np.asarray(inputs["W_prior"], np.float32))
    b_prior = np.asarray(inputs["b_prior"], np.float32)
    gate = np.asarray(inputs["gate"], np.float32)

    chunks, B_wb, cblk, midx_p, tidx_p, dmod_c, dstf_c = _prep_edges(edge_index)

    iota_g = np.broadcast_to(
        np.tile(np.arange(P, dtype=np.float32), GMAX)[None, :],
        (P, GMAX * P)).copy()
    io16_g = np.broadcast_to(
        np.tile(np.arange(16, dtype=np.float32), GMAX)[None, :],
        (P, GMAX * 16)).copy()
    shared = {
        "x": x, "wg": wg, "wp": wp,
        "attsrc_b": np.broadcast_to(att_src.reshape(1, HF), (P, HF)).copy(),
        "attdst_b": np.broadcast_to(att_dst.reshape(1, HF), (P, HF)).copy(),
        "biasg_b": np.broadcast_to(bias_gat.reshape(1, F), (P, F)).copy(),
        "bprior_b": np.broadcast_to(b_prior.reshape(1, F), (P, F)).copy(),
        "gate_b": np.broadcast_to(gate.reshape(1, 1), (P, 1)).copy(),
        "iota_g": iota_g, "iota16_g": io16_g,
    }
    in_maps = []
    for c in range(NCORES):
        m = dict(shared)
        m["midx"] = midx_p[c]
        m["tidx"] = tidx_p[c]
        m["dmod"] = dmod_c[c]
        m["dstf"] = dstf_c[c]
        in_maps.append(m)

    for c in range(NCORES):
        in_maps[c]["kg"] = np.ascontiguousarray(kg[c * SH:(c + 1) * SH])

    nc = _build_nc(chunks, cblk)
    res = run_bass_kernel_spmd(nc, in_maps, core_ids=list(range(NCORES)))
    out = np.concatenate([res.results[c]["out"] for c in range(NCORES)], axis=0)
    return out.astype(np.float32)



# revision 8
# speedup vs baseline: 1.2064x; 1.2064x over previous
"""Trainium2 Bass kernel for ClimateConditionedGAT (GATConv + one-hot prior gate).

Strategy (8 NeuronCores, SPMD single NEFF):
  - Nodes sharded by destination: core c owns dst nodes [c*6250, (c+1)*6250).
  - Head-interleaved feature layout: h columns are (j, head) pairs, i.e.
    col 4*j+hd = h[hd*64+j]. This makes the per-edge ex broadcast and the
    one-hot compare packed in their last AP dim, unlocking the DVE 2x_1p
    mode (2-byte dtypes).
  - Phase 1 (replicated on every core): h = x @ W_ext from a host-transposed
    bf16 xT (no PE transpose). Per 128-node block the [128,264] f32 PSUM
    result yields:
      * T-row (260 bf16 = 520B payload in a 768B-stride row): [h(256)|a_src(4)]
        written in batches of 15 blocks per DMA; two 25001-row source buckets
        (dma_gather indices are int16), each with a sentinel row
        (a_src = -1e30 => exp -> 0).
      * a_dst (4 f32) staged and dma_scatter_add'ed into a per-core LOCAL
        A-table [6272, 64] f32 via host-provided indices (own dst rows at
        0..6249, everything else onto trash row 6250), so phase 2 can fetch
        a_dst with a direct int16 gather.
  - Phase 2: edges grouped by (dst-window of 128, src bucket), padded to the
    max count over cores (16-granular; shared instruction stream). Per group:
    one 768B-row dma_gather of T by src and one 256B-row dma_gather of A by
    local dst; ex = exp(leaky(asrc+adst)); S[e,d] = (dstf==d) (bf16, 2x) and
    rhs = [ex*h | ex] (bf16, 2x) feed PE bf16 matmuls accumulating
    [sum ex*h | sum ex] into a per-window PSUM tile; the window epilogue
    normalizes, averages heads, adds bias, computes prior = kg @ W_prior
    (host-transposed kgT, single preload) and gates the two; outputs are
    staged and written 8 windows per DMA.
All float math on device; host does integer edge partitioning/packing and
dtype/layout conversion of inputs only.
"""
import sys
import numpy as np

if "/opt/trn_rl_repo" not in sys.path:
    sys.path.insert(0, "/opt/trn_rl_repo")

from contextlib import ExitStack

import ml_dtypes

import concourse.bass as bass
import concourse.bacc as bacc
import concourse.mybir as mybir
import concourse.tile as tile
from concourse.bass_utils import run_bass_kernel_spmd

BF16 = ml_dtypes.bfloat16

P = 128
N = 50000
DIN = 128
HEADS = 4
F = 64
HF = HEADS * F          # 256
NCORES = 8
SH = N // NCORES        # 6250 dst nodes per core
ROWE = 384              # T row stride in bf16 elems (768 B)
ROWD = 260              # written part of a T row (520 B): h(256) + asrc(4)
BUCK_N = 25000          # real nodes per source bucket
BROWS = BUCK_N + 1      # bucket rows incl sentinel
SENT = BUCK_N           # bucket-local sentinel index
NB = (BUCK_N + P - 1) // P      # 196 phase-1 blocks per bucket
NBF = BUCK_N // P               # 195 full blocks
WIN = P                 # dst window size
NWIN = (SH + WIN - 1) // WIN    # 49 windows per core
AROWS = NWIN * P        # A-table rows (6272); trash row = SH
ATRASH = SH
NXPAD = 2 * NB * P      # padded xT columns (50176)
ACH = 32                # blocks of a_dst staged per scatter
TB = 15                 # full blocks of T rows staged per write
XB = 8                  # blocks of x loaded per DMA
OB = 8                  # windows of output staged per write
NEG_SLOPE = 0.2
dt = mybir.dt
Alu = mybir.AluOpType
Act = mybir.ActivationFunctionType


# ----------------------------------------------------------------------------
# host-side edge preparation (integer/layout work only)
# ----------------------------------------------------------------------------

def _pack_idx16(idx):
    """Pack int array -> [128, ceil(n/16)] int16 (wrap 16, replicate x8)."""
    n = len(idx)
    cols = (n + 15) // 16
    a = np.zeros((16, cols), np.int16)
    a[np.arange(n) % 16, np.arange(n) // 16] = idx.astype(np.int16)
    return np.tile(a, (8, 1))


def _prep_edges(edge_index):
    src = np.asarray(edge_index[0], dtype=np.int64)
    dst = np.asarray(edge_index[1], dtype=np.int64)
    loops = np.arange(N, dtype=np.int64)
    src = np.concatenate([src, loops])
    dst = np.concatenate([dst, loops])

    core = dst // SH
    dl = dst % SH
    w = dl // WIN
    b = (src >= BUCK_N).astype(np.int64)

    gid = (core * NWIN + w) * 2 + b
    ngroups = NCORES * NWIN * 2
    counts = np.bincount(gid, minlength=ngroups).reshape(NCORES, NWIN, 2)

    # uniform counts across cores (shared NEFF), 16-granular
    n_wb = counts.max(axis=0)                        # [NWIN, 2]
    n16 = np.maximum(((n_wb + P - 1) // P) * P, P)   # [NWIN, 2]
    g_wb = (n16 + P - 1) // P                        # blocks per group
    gmax = int(g_wb.max())
    s16 = int(n16.sum())                             # total gathered slots
    cblk = int(g_wb.sum())                           # total blocks

    # chunk list: (w, g, blk0, ic0, nidx, tbase, first, last) in emission
    # order; groups larger than GSPL blocks are split so one dma_gather never
    # exceeds GSPL*128 indices (SWDGE descriptor ring capacity)
    GSPL = 8
    chunks = []
    ic = 0
    blk0 = 0
    for wi in range(NWIN):
        for bi in range(2):
            rem = int(g_wb[wi, bi])
            while rem > 0:
                gg = min(rem, GSPL)
                first = bi == 0 and rem == int(g_wb[wi, 0])
                last = bi == 1 and rem == gg
                chunks.append((wi, gg, blk0, ic // 16, gg * P,
                               bi * BROWS, first, last))
                ic += gg * P
                blk0 += gg
                rem -= gg
    gmax = min(gmax, GSPL)

    order = np.argsort(gid, kind="stable")
    src_s, dst_s = src[order], dst[order]
    grp_starts = np.zeros(ngroups + 1, np.int64)
    np.cumsum(np.bincount(gid[order], minlength=ngroups), out=grp_starts[1:])

    s_off = np.zeros((NWIN, 2), np.int64)
    b_off = np.zeros((NWIN, 2), np.int64)
    o = 0
    ob = 0
    for wi in range(NWIN):
        for bi in range(2):
            s_off[wi, bi] = o
            b_off[wi, bi] = ob
            o += int(n16[wi, bi])
            ob += int(g_wb[wi, bi])

    midx = np.full((NCORES, s16), SENT, np.int64)    # bucket-local src row
    aidx = np.full((NCORES, s16), ATRASH, np.int64)  # core-local dst row
    dstf = np.full((NCORES, cblk, P), -1.0, np.float32)  # dst local to window

    for c in range(NCORES):
        for wi in range(NWIN):
            for bi in range(2):
                gsl = (c * NWIN + wi) * 2 + bi
                a0, a1 = grp_starts[gsl], grp_starts[gsl + 1]
                n = int(a1 - a0)
                if n == 0:
                    continue
                o = int(s_off[wi, bi])
                es, ed = src_s[a0:a1], dst_s[a0:a1]
                edl = ed % SH
                midx[c, o:o + n] = es - bi * BUCK_N
                aidx[c, o:o + n] = edl
                sl = np.arange(n)
                blk = int(b_off[wi, bi]) + sl // P
                dstf[c, blk, sl % P] = (edl - wi * WIN).astype(np.float32)

    midx_p = np.stack([_pack_idx16(midx[c]) for c in range(NCORES)])
    aidx_p = np.stack([_pack_idx16(aidx[c]) for c in range(NCORES)])
    # dstf2: each value duplicated along a new inner dim of 2 (packed last
    # dim for the DVE 2x one-hot compare): [P, cblk*2]
    dstf2 = np.repeat(dstf.transpose(0, 2, 1), 2, axis=2).astype(BF16)

    # phase-1 a_dst scatter indices: slot = (bkt*NB + bi)*128 + p
    sl = np.arange(2 * NB * P)
    bkt = sl // (NB * P)
    off = sl % (NB * P)
    node = bkt * BUCK_N + off
    valid = (off < BUCK_N) & (node < N)
    sidx = np.full((NCORES, 2 * NB * P), ATRASH, np.int64)
    for c in range(NCORES):
        owned = valid & (node // SH == c)
        sidx[c] = np.where(owned, node - c * SH, ATRASH)
    sidx_p = np.stack([_pack_idx16(sidx[c]) for c in range(NCORES)])

    return chunks, gmax, cblk, s16, midx_p, aidx_p, dstf2, sidx_p


# ----------------------------------------------------------------------------
# device program
# ----------------------------------------------------------------------------

def _build_nc(chunks, gmax, cblk, s16):
    nc = bacc.Bacc("TRN2", target_bir_lowering=False, debug=False,
                   dynamic_dma_scratch_size=16384)

    t_xt = nc.dram_tensor("xt", [DIN, NXPAD], dt.bfloat16, kind="ExternalInput")
    t_wg = nc.dram_tensor("wg", [DIN, HF], dt.float32, kind="ExternalInput")
    t_asb = nc.dram_tensor("attsrc_b", [P, HF], dt.float32, kind="ExternalInput")
    t_adb = nc.dram_tensor("attdst_b", [P, HF], dt.float32, kind="ExternalInput")
    t_bg = nc.dram_tensor("biasg_b", [P, F], dt.float32, kind="ExternalInput")
    t_bp = nc.dram_tensor("bprior_b", [P, F], dt.float32, kind="ExternalInput")
    t_gate = nc.dram_tensor("gate_b", [P, 1], dt.float32, kind="ExternalInput")
    t_wp = nc.dram_tensor("wp16", [32, F], dt.bfloat16, kind="ExternalInput")
    t_kgt = nc.dram_tensor("kgt", [32, AROWS], dt.bfloat16, kind="ExternalInput")
    t_iota = nc.dram_tensor("iota_g", [P, gmax * P], dt.bfloat16,
                            kind="ExternalInput")
    t_midx = nc.dram_tensor("midx", [P, s16 // 16], dt.int16,
                            kind="ExternalInput")
    t_aidx = nc.dram_tensor("aidx", [P, s16 // 16], dt.int16,
                            kind="ExternalInput")
    t_dstf = nc.dram_tensor("dstf2", [P, cblk * 2], dt.bfloat16,
                            kind="ExternalInput")
    t_sidx = nc.dram_tensor("sidx", [P, (2 * NB * P) // 16], dt.int16,
                            kind="ExternalInput")
    t_out = nc.dram_tensor("out", [SH, F], dt.float32, kind="ExternalOutput")

    t_T = nc.dram_tensor("tbl", [2 * BROWS, ROWE], dt.bfloat16)
    t_A = nc.dram_tensor("adst", [AROWS, F], dt.float32)

    def v3(t, d1, s1, d2, s2):
        return bass.AP(t.tensor, t.offset, [t.ap[0], [s1, d1], [s2, d2]])

    def v4(t, d1, s1, d2, s2, d3, s3, off=0):
        return bass.AP(t.tensor, t.offset + off,
                       [t.ap[0], [s1, d1], [s2, d2], [s3, d3]])

    with tile.TileContext(nc) as tc:
        with ExitStack() as octx:
            cp = octx.enter_context(tc.tile_pool(name="const", bufs=1))

            iota_sb = cp.tile([P, gmax * P], dt.bfloat16)
            nc.sync.dma_start(iota_sb[:], t_iota[:])
            wp_sb = cp.tile([32, F], dt.bfloat16)
            nc.sync.dma_start(wp_sb[:], t_wp[:])
            kgt_sb = cp.tile([32, AROWS], dt.bfloat16)
            nc.sync.dma_start(kgt_sb[:], t_kgt[:])
            asb = cp.tile([P, HF], dt.float32)
            nc.sync.dma_start(asb[:], t_asb[:])
            adb = cp.tile([P, HF], dt.float32)
            nc.sync.dma_start(adb[:], t_adb[:])
            bg_sb = cp.tile([P, F], dt.float32)
            nc.sync.dma_start(bg_sb[:], t_bg[:])
            bp_sb = cp.tile([P, F], dt.float32)
            nc.sync.dma_start(bp_sb[:], t_bp[:])
            gate_sb = cp.tile([P, 1], dt.float32)
            nc.sync.dma_start(gate_sb[:], t_gate[:])
            g1m_sb = cp.tile([P, 1], dt.float32)
            nc.vector.tensor_scalar(g1m_sb[:], gate_sb[:], -1.0, 1.0,
                                    op0=Alu.mult, op1=Alu.add)
            midx_sb = cp.tile([P, s16 // 16], dt.int16)
            nc.sync.dma_start(midx_sb[:], t_midx[:])
            aidx_sb = cp.tile([P, s16 // 16], dt.int16)
            nc.sync.dma_start(aidx_sb[:], t_aidx[:])
            dstf_sb = cp.tile([P, cblk * 2], dt.bfloat16)
            nc.sync.dma_start(dstf_sb[:], t_dstf[:])
            sidx_sb = cp.tile([P, (2 * NB * P) // 16], dt.int16)
            nc.sync.dma_start(sidx_sb[:], t_sidx[:])

            # W_ext = [W_gat | wsrc(4) | wdst(4)] in (j, head) interleave;
            # wsrc_hd = sum_j W[:,(j,hd)]*att_src[(j,hd)]
            wext = cp.tile([P, HF + 8], dt.float32)
            nc.sync.dma_start(wext[:, 0:HF], t_wg[:])
            with tc.tile_pool(name="wtmp", bufs=2) as wtp:
                for j in range(HEADS):
                    for k, att in enumerate((asb, adb)):
                        tmp = wtp.tile([P, F], dt.float32, tag="wtmp")
                        nc.vector.tensor_tensor(
                            out=tmp[:],
                            in0=v3(bass.AP(wext[:].tensor, wext[:].offset + j,
                                           [wext[:].ap[0]]), F, 4, 1, 1),
                            in1=v3(bass.AP(att[:].tensor, att[:].offset + j,
                                           [att[:].ap[0]]), F, 4, 1, 1),
                            op=Alu.mult)
                        nc.vector.tensor_reduce(
                            out=wext[:, HF + 4 * k + j:HF + 4 * k + j + 1],
                            in_=tmp[:], axis=mybir.AxisListType.X, op=Alu.add)
            wext16 = cp.tile([P, HF + 8], dt.bfloat16)
            nc.vector.tensor_copy(out=wext16[:], in_=wext[:])

            # zero-fill the A-table (scatter_add target)
            zt = cp.tile([P, NWIN * F], dt.float32)
            nc.vector.memset(zt[:], 0.0)
            nc.sync.dma_start(
                t_A[:, :].rearrange("(p a) c -> p (a c)", p=P), zt[:])

            # ---------------- phase 1: build T and A ----------------
            with tc.tile_pool(name="p1x", bufs=2) as p1x, \
                 tc.tile_pool(name="p1r", bufs=2) as p1r, \
                 tc.tile_pool(name="p1s", bufs=2) as p1s, \
                 tc.tile_pool(name="p1ps", bufs=4, space="PSUM") as pp1, \
                 tc.tile_pool(name="p1ac", bufs=2) as p1a:
                xb = None
                rb = None
                ach_tile = None
                for bkt in range(2):
                    for bi in range(NB):
                        if bi % XB == 0:
                            colb = bkt * BUCK_N + bi * P
                            nx = min(XB, NB - bi) * P
                            xb = p1x.tile([P, XB * P], dt.bfloat16, tag="xb")
                            nc.sync.dma_start(xb[:, 0:nx],
                                              t_xt[:, colb:colb + nx])
                        h_ps = pp1.tile([P, HF + 8], dt.float32, space="PSUM",
                                        tag="hps")
                        nc.tensor.matmul(
                            h_ps[:],
                            lhsT=xb[:, (bi % XB) * P:(bi % XB + 1) * P],
                            rhs=wext16[:], start=True, stop=True)
                        # T-row staging (TB full blocks per write)
                        full = bi < NBF
                        if full:
                            ti = (bi % TB)
                            if ti == 0:
                                rb = p1r.tile([P, TB, ROWD], dt.bfloat16,
                                              tag="rb")
                            nc.vector.tensor_copy(out=rb[:, ti, 0:P],
                                                  in_=h_ps[:, 0:P])
                            nc.scalar.copy(rb[:, ti, P:ROWD],
                                           h_ps[:, P:ROWD])
                            if ti == TB - 1 or bi == NBF - 1:
                                nb = ti + 1
                                r0 = bkt * BROWS + (bi - ti) * P
                                dst_ap = t_T[r0:r0 + nb * P, 0:ROWD] \
                                    .rearrange("(b p) c -> p b c", p=P)
                                nc.sync.dma_start(dst_ap, rb[:, 0:nb, :])
                        else:
                            # partial tail block: write its rows directly
                            nr = BUCK_N - bi * P
                            rt = p1s.tile([P, ROWD], dt.bfloat16, tag="rt")
                            nc.vector.tensor_copy(out=rt[:, 0:P],
                                                  in_=h_ps[:, 0:P])
                            nc.scalar.copy(rt[:, P:ROWD], h_ps[:, P:ROWD])
                            nc.sync.dma_start(
                                t_T[bkt * BROWS + bi * P:
                                    bkt * BROWS + bi * P + nr, 0:ROWD],
                                rt[:nr, :])
                        ci = bi % ACH
                        if ci == 0:
                            ach_tile = p1a.tile([P, ACH, HEADS], dt.float32,
                                                tag="ach")
                        nc.vector.tensor_copy(out=ach_tile[:, ci, :],
                                              in_=h_ps[:, HF + 4:HF + 8])
                        if ci == ACH - 1 or bi == NB - 1:
                            nb = ci + 1
                            slot0 = (bkt * NB + bi - ci) * P
                            nc.gpsimd.dma_scatter_add(
                                t_A[:, 0:HEADS], ach_tile[:, 0:nb, :],
                                sidx_sb[:, slot0 // 16:(slot0 + nb * P) // 16],
                                nb * P, nb * P, HEADS, F)
                # sentinel rows (h=0, a_src=-1e30)
                sent = p1s.tile([1, ROWD], dt.bfloat16, tag="sent")
                nc.gpsimd.memset(sent[:], 0.0)
                nc.gpsimd.memset(sent[:, HF:ROWD], -1e30)
                nc.sync.dma_start(t_T[SENT:SENT + 1, 0:ROWD], sent[:])
                nc.sync.dma_start(t_T[BROWS + SENT:BROWS + SENT + 1, 0:ROWD],
                                  sent[:])

            # ---------------- phase 2: gather / scatter ----------------
            with tc.tile_pool(name="g1p", bufs=3) as g1p, \
                 tc.tile_pool(name="g2p", bufs=3) as g2p, \
                 tc.tile_pool(name="wk", bufs=3) as wk, \
                 tc.tile_pool(name="sp", bufs=3) as sp, \
                 tc.tile_pool(name="accp", bufs=3, space="PSUM") as accp, \
                 tc.tile_pool(name="prp", bufs=2, space="PSUM") as prp, \
                 tc.tile_pool(name="fin", bufs=3) as fin, \
                 tc.tile_pool(name="outp", bufs=2) as outp:

                acc_ps = None
                ob_tile = None
                for (wi, g, blk0, ic0, nn, tbase, first, last) in chunks:
                    if first:
                        acc_ps = accp.tile([P, HF + 4], dt.float32,
                                           space="PSUM", tag="acc")
                    g1 = g1p.tile([P, gmax, ROWE], dt.bfloat16, tag="g1")
                    nc.gpsimd.dma_gather(
                        g1[:, 0:g, :], t_T[tbase:tbase + BROWS, :],
                        midx_sb[:, ic0:ic0 + nn // 16], nn, nn, ROWE)
                    g2 = g2p.tile([P, gmax, F], dt.float32, tag="g2")
                    nc.gpsimd.dma_gather(
                        g2[:, 0:g, :], t_A[:, :],
                        aidx_sb[:, ic0:ic0 + nn // 16], nn, nn, F)

                    g1a = g1[:, 0:g, :]
                    # alpha = leaky(asrc + adst)
                    alpha = wk.tile([P, gmax * HEADS], dt.float32, tag="alpha")
                    nc.vector.tensor_tensor(
                        out=v3(alpha[:], g, 4, 4, 1),
                        in0=bass.AP(g1a.tensor, g1a.offset + HF,
                                    [g1a.ap[0], [ROWE, g], [1, 4]]),
                        in1=bass.AP(g2[:].tensor, g2[:].offset,
                                    [g2[:].ap[0], [F, g], [1, 4]]),
                        op=Alu.add)
                    nc.vector.scalar_tensor_tensor(
                        out=alpha[:, 0:g * 4], in0=alpha[:, 0:g * 4],
                        scalar=NEG_SLOPE, in1=alpha[:, 0:g * 4],
                        op0=Alu.mult, op1=Alu.max)
                    ex = wk.tile([P, gmax * HEADS], dt.bfloat16, tag="ex")
                    nc.scalar.activation(ex[:, 0:g * 4], alpha[:, 0:g * 4],
                                         Act.Exp)
                    # S[e, d] = (dstf == d)  [P, g, 128] bf16, 2x via dup'd
                    # dstf and (d1,d0) split so every last dim is packed
                    S = sp.tile([P, gmax * P], dt.bfloat16, tag="S")
                    nc.vector.tensor_tensor(
                        out=v4(S[:], g, P, F, 2, 2, 1),
                        in0=v4(dstf_sb[:, blk0 * 2:(blk0 + g) * 2],
                               g, 2, F, 0, 2, 1),
                        in1=v4(iota_sb[:, 0:g * P], g, 0, F, 2, 2, 1),
                        op=Alu.is_equal)
                    # rhs = [ex*h (256, (j,hd) order) | ex (4)]  [P, g, 260]
                    rhs = wk.tile([P, gmax * (HF + 4)], dt.bfloat16, tag="rhs")
                    nc.vector.tensor_tensor(
                        out=v4(rhs[:], g, HF + 4, F, 4, 4, 1),
                        in0=v4(g1a, g, ROWE, F, 4, 4, 1),
                        in1=v4(ex[:], g, 4, F, 0, 4, 1),
                        op=Alu.mult)
                    nc.scalar.copy(
                        bass.AP(rhs[:].tensor, rhs[:].offset + HF,
                                [rhs[:].ap[0], [HF + 4, g], [1, 4]]),
                        v3(ex[:], g, 4, 4, 1))
                    for k in range(g):
                        nc.tensor.matmul(
                            acc_ps[:],
                            lhsT=S[:, k * P:(k + 1) * P],
                            rhs=rhs[:, k * (HF + 4):(k + 1) * (HF + 4)],
                            start=(first and k == 0),
                            stop=(last and k == g - 1))

                    if last:
                        # ---- window epilogue (acc cols are (j, hd)) ----
                        nw = min(WIN, SH - wi * WIN)
                        den = fin.tile([P, HEADS], dt.float32, tag="den")
                        nc.vector.tensor_scalar(
                            den[:], acc_ps[:, HF:HF + 4], 1e-16, float(HEADS),
                            op0=Alu.add, op1=Alu.mult)
                        rec = fin.tile([P, HEADS], dt.float32, tag="rec")
                        nc.vector.reciprocal(rec[:], den[:])
                        prod = fin.tile([P, HF], dt.float32, tag="prod")
                        nc.vector.tensor_tensor(
                            out=v4(prod[:], F, 4, 1, 0, 4, 1),
                            in0=v4(acc_ps[:, 0:HF], F, 4, 1, 0, 4, 1),
                            in1=v4(rec[:], F, 0, 1, 0, 4, 1), op=Alu.mult)
                        oi = wi % OB
                        if oi == 0:
                            ob_tile = outp.tile([P, OB, F], dt.float32,
                                                tag="ot")
                        gat = fin.tile([P, F], dt.float32, tag="gat")
                        nc.vector.tensor_reduce(
                            out=gat[:],
                            in_=v3(prod[:], F, 4, HEADS, 1),
                            axis=mybir.AxisListType.X, op=Alu.add)
                        # prior = kgT_win.T @ W_prior
                        pr_ps = prp.tile([P, F], dt.float32, space="PSUM",
                                         tag="prps")
                        nc.tensor.matmul(pr_ps[:],
                                         lhsT=kgt_sb[:, wi * P:(wi + 1) * P],
                                         rhs=wp_sb[:], start=True, stop=True)
                        # combine: (1-g)*(gat+bias) + g*(prior+bprior)
                        t1 = ob_tile[:, oi, :]
                        nc.vector.tensor_tensor(out=t1, in0=gat[:],
                                                in1=bg_sb[:], op=Alu.add)
                        nc.vector.tensor_scalar_mul(t1, t1, g1m_sb[:, 0:1])
                        t2 = fin.tile([P, F], dt.float32, tag="t2")
                        nc.vector.tensor_tensor(out=t2[:], in0=pr_ps[:],
                                                in1=bp_sb[:], op=Alu.add)
                        nc.vector.tensor_scalar_mul(t2[:], t2[:],
                                                    gate_sb[:, 0:1])
                        nc.vector.tensor_tensor(out=t1, in0=t1, in1=t2[:],
                                                op=Alu.add)
                        if oi == OB - 1 or wi == NWIN - 1:
                            nbw = oi + 1
                            w0 = wi - oi
                            nrows = min(OB * P, SH - w0 * P)
                            dst_ap = t_out[w0 * P:w0 * P + nrows, :] \
                                .rearrange("(b p) c -> p b c", p=P) \
                                if nrows == nbw * P else None
                            if dst_ap is not None:
                                nc.sync.dma_start(dst_ap, ob_tile[:, 0:nbw, :])
                            else:
                                # tail: full windows then the partial one
                                if nbw > 1:
                                    da = t_out[w0 * P:w0 * P + (nbw - 1) * P,
                                               :].rearrange(
                                        "(b p) c -> p b c", p=P)
                                    nc.sync.dma_start(da,
                                                      ob_tile[:, 0:nbw - 1, :])
                                nc.sync.dma_start(
                                    t_out[wi * P:wi * P + nw, :],
                                    ob_tile[:nw, nbw - 1, :])

    nc.compile()
    return nc


def _interleave_cols(a):
    """[..., HEADS*F] head-major -> (j, head) interleaved columns."""
    s = a.shape[:-1]
    return np.ascontiguousarray(
        a.reshape(s + (HEADS, F)).swapaxes(-1, -2).reshape(s + (HEADS * F,)))


def kernel(**inputs):
    x = np.asarray(inputs["x"], np.float32)
    edge_index = np.asarray(inputs["edge_index"])
    kg = np.asarray(inputs["kg_onehot"], np.float32)
    wg = np.ascontiguousarray(np.asarray(inputs["W_gat"], np.float32))
    att_src = np.asarray(inputs["att_src"], np.float32)
    att_dst = np.asarray(inputs["att_dst"], np.float32)
    bias_gat = np.asarray(inputs["bias_gat"], np.float32)
    wp = np.asarray(inputs["W_prior"], np.float32)
    b_prior = np.asarray(inputs["b_prior"], np.float32)
    gate = np.asarray(inputs["gate"], np.float32)

    chunks, gmax, cblk, s16, midx_p, aidx_p, dstf2, sidx_p = \
        _prep_edges(edge_index)

    xt = np.zeros((DIN, NXPAD), BF16)
    xt[:, 0:N] = np.ascontiguousarray(x.T).astype(BF16)
    iota_g = np.broadcast_to(
        np.tile(np.arange(P, dtype=np.float32), gmax)[None, :],
        (P, gmax * P)).astype(BF16)

    wg_i = _interleave_cols(wg)                      # [128, 256] (j, hd)
    att_src_i = _interleave_cols(att_src.reshape(1, HF))
    att_dst_i = _interleave_cols(att_dst.reshape(1, HF))

    shared = {
        "xt": xt, "wg": wg_i,
        "wp16": wp.astype(BF16),
        "attsrc_b": np.broadcast_to(att_src_i, (P, HF)).copy(),
        "attdst_b": np.broadcast_to(att_dst_i, (P, HF)).copy(),
        "biasg_b": np.broadcast_to(bias_gat.reshape(1, F), (P, F)).copy(),
        "bprior_b": np.broadcast_to(b_prior.reshape(1, F), (P, F)).copy(),
        "gate_b": np.broadcast_to(gate.reshape(1, 1), (P, 1)).copy(),
        "iota_g": np.ascontiguousarray(iota_g),
    }
    in_maps = []
    for c in range(NCORES):
        m = dict(shared)
        m["midx"] = midx_p[c]
        m["aidx"] = aidx_p[c]
        m["dstf2"] = dstf2[c]
        m["sidx"] = sidx_p[c]
        kgt = np.zeros((32, AROWS), BF16)
        kgt[:, 0:SH] = kg[c * SH:(c + 1) * SH].T.astype(BF16)
        m["kgt"] = kgt
        in_maps.append(m)

    nc = _build_nc(chunks, gmax, cblk, s16)
    res = run_bass_kernel_spmd(nc, in_maps, core_ids=list(range(NCORES)))
    out = np.concatenate([res.results[c]["out"] for c in range(NCORES)],
                         axis=0)
    return out.astype(np.float32)


# revision 17
# speedup vs baseline: 1.4518x; 1.2034x over previous
"""Trainium2 Bass kernel for ClimateConditionedGAT (GATConv + one-hot prior gate).

Strategy (8 NeuronCores, SPMD single NEFF):
  - Nodes sharded by destination: core c owns dst nodes [c*6250, (c+1)*6250).
  - Head-interleaved feature layout: h columns are (j, head) pairs, i.e.
    col 4*j+hd = h[hd*64+j]. This makes the per-edge ex broadcast and the
    one-hot compare packed in their last AP dim, unlocking the DVE 2x_1p
    mode (2-byte dtypes).
  - Phase 1 (replicated on every core): h = x @ W_ext from a host-transposed
    bf16 xT (no PE transpose). Per 128-node block the [128,264] f32 PSUM
    result yields:
      * T-row (260 bf16 = 520B payload in a 768B-stride row): [h(256)|a_src(4)]
        written in batches of 15 blocks per DMA; two 25001-row source buckets
        (dma_gather indices are int16), each with a sentinel row
        (a_src = -1e30 => exp -> 0).
      * a_dst (4 f32) staged and dma_scatter_add'ed into a per-core LOCAL
        A-table [6272, 64] f32 via host-provided indices (own dst rows at
        0..6249, everything else onto trash row 6250), so phase 2 can fetch
        a_dst with a direct int16 gather.
  - Phase 2: edges grouped by (dst-window of 128, src bucket), padded to the
    max count over cores (16-granular; shared instruction stream). Per group:
    one 768B-row dma_gather of T by src and one 256B-row dma_gather of A by
    local dst; ex = exp(leaky(asrc+adst)); S[e,d] = (dstf==d) (bf16, 2x) and
    rhs = [ex*h | ex] (bf16, 2x) feed PE bf16 matmuls accumulating
    [sum ex*h | sum ex] into a per-window PSUM tile; the window epilogue
    normalizes, averages heads, adds bias, computes prior = kg @ W_prior
    (host-transposed kgT, single preload) and gates the two; outputs are
    staged and written 8 windows per DMA.
All float math on device; host does integer edge partitioning/packing and
dtype/layout conversion of inputs only.
"""
import sys
import numpy as np

if "/opt/trn_rl_repo" not in sys.path:
    sys.path.insert(0, "/opt/trn_rl_repo")

from contextlib import ExitStack

import ml_dtypes

import concourse.bass as bass
import concourse.bacc as bacc
import concourse.mybir as mybir
import concourse.tile as tile
from concourse.bass_utils import run_bass_kernel_spmd

BF16 = ml_dtypes.bfloat16

P = 128
N = 50000
DIN = 128
HEADS = 4
F = 64
HF = HEADS * F          # 256
NCORES = 8
SH = N // NCORES        # 6250 dst nodes per core
ROWE = 384              # T row stride in bf16 elems (768 B)
ROWD = 260              # written part of a T row (520 B): h(256) + asrc(4)
BUCK_N = 25000          # real nodes per source bucket
BROWS = BUCK_N + 1      # bucket rows incl sentinel
SENT = BUCK_N           # bucket-local sentinel index
NB = (BUCK_N + P - 1) // P      # 196 phase-1 blocks per bucket
NBF = BUCK_N // P               # 195 full blocks
WIN = 96                # dst window size (<=128; sized so one (w,b) group
                        # usually fits a single 1024-index dma_gather)
NWIN = (SH + WIN - 1) // WIN    # 66 windows per core
AROWS = 6272            # A-table rows; trash row = SH
KGTC = NWIN * WIN + P   # kgt padded cols (window slices are 128 wide)
ATRASH = SH
NXPAD = 2 * NB * P      # padded xT columns (50176)
ACH = 32                # blocks of a_dst staged per scatter
TB = 15                 # full blocks of T rows staged per write
XB = 8                  # blocks of x loaded per DMA
OB = 8                  # windows of output staged per write
NEG_SLOPE = 0.2
dt = mybir.dt
Alu = mybir.AluOpType
Act = mybir.ActivationFunctionType


# ----------------------------------------------------------------------------
# host-side edge preparation (integer/layout work only)
# ----------------------------------------------------------------------------

def _pack_idx16(idx):
    """Pack int array -> [128, ceil(n/16)] int16 (wrap 16, replicate x8)."""
    n = len(idx)
    cols = (n + 15) // 16
    a = np.zeros((16, cols), np.int16)
    a[np.arange(n) % 16, np.arange(n) // 16] = idx.astype(np.int16)
    return np.tile(a, (8, 1))


def _prep_edges(edge_index):
    src = np.asarray(edge_index[0], dtype=np.int64)
    dst = np.asarray(edge_index[1], dtype=np.int64)
    loops = np.arange(N, dtype=np.int64)
    src = np.concatenate([src, loops])
    dst = np.concatenate([dst, loops])

    core = dst // SH
    dl = dst % SH
    w = dl // WIN
    b = (src >= BUCK_N).astype(np.int64)

    gid = (core * NWIN + w) * 2 + b
    ngroups = NCORES * NWIN * 2
    counts = np.bincount(gid, minlength=ngroups).reshape(NCORES, NWIN, 2)

    # uniform counts across cores (shared NEFF), 16-granular
    n_wb = counts.max(axis=0)                        # [NWIN, 2]
    n16 = np.maximum(((n_wb + P - 1) // P) * P, P)   # [NWIN, 2]
    g_wb = (n16 + P - 1) // P                        # blocks per group
    gmax = int(g_wb.max())
    s16 = int(n16.sum())                             # total gathered slots
    cblk = int(g_wb.sum())                           # total blocks

    # chunk list: (w, g, blk0, ic0, nidx, tbase, first, last) in emission
    # order; groups larger than GSPL blocks are split so one dma_gather never
    # exceeds GSPL*128 indices (SWDGE descriptor ring capacity)
    GSPL = 8
    chunks = []
    ic = 0
    blk0 = 0
    for wi in range(NWIN):
        for bi in range(2):
            rem = int(g_wb[wi, bi])
            while rem > 0:
                gg = min(rem, GSPL)
                first = bi == 0 and rem == int(g_wb[wi, 0])
                last = bi == 1 and rem == gg
                chunks.append((wi, gg, blk0, ic // 16, gg * P,
                               bi * BROWS, first, last))
                ic += gg * P
                blk0 += gg
                rem -= gg
    gmax = min(gmax, GSPL)

    order = np.argsort(gid, kind="stable")
    src_s, dst_s = src[order], dst[order]
    grp_starts = np.zeros(ngroups + 1, np.int64)
    np.cumsum(np.bincount(gid[order], minlength=ngroups), out=grp_starts[1:])

    s_off = np.zeros((NWIN, 2), np.int64)
    b_off = np.zeros((NWIN, 2), np.int64)
    o = 0
    ob = 0
    for wi in range(NWIN):
        for bi in range(2):
            s_off[wi, bi] = o
            b_off[wi, bi] = ob
            o += int(n16[wi, bi])
            ob += int(g_wb[wi, bi])

    midx = np.full((NCORES, s16), SENT, np.int64)    # bucket-local src row
    aidx = np.full((NCORES, s16), ATRASH, np.int64)  # core-local dst row
    dstf = np.full((NCORES, cblk, P), -1.0, np.float32)  # dst local to window

    for c in range(NCORES):
        for wi in range(NWIN):
            for bi in range(2):
                gsl = (c * NWIN + wi) * 2 + bi
                a0, a1 = grp_starts[gsl], grp_starts[gsl + 1]
                n = int(a1 - a0)
                if n == 0:
                    continue
                o = int(s_off[wi, bi])
                es, ed = src_s[a0:a1], dst_s[a0:a1]
                edl = ed % SH
                midx[c, o:o + n] = es - bi * BUCK_N
                aidx[c, o:o + n] = edl
                sl = np.arange(n)
                blk = int(b_off[wi, bi]) + sl // P
                dstf[c, blk, sl % P] = (edl - wi * WIN).astype(np.float32)

    midx_p = np.stack([_pack_idx16(midx[c]) for c in range(NCORES)])
    aidx_p = np.stack([_pack_idx16(aidx[c]) for c in range(NCORES)])
    # dstf2: each value duplicated along a new inner dim of 2 (packed last
    # dim for the DVE 2x one-hot compare): [P, cblk*2]
    dstf2 = np.repeat(dstf.transpose(0, 2, 1), 2, axis=2).astype(BF16)

    # phase-1 a_dst scatter indices: slot = (bkt*NB + bi)*128 + p
    sl = np.arange(2 * NB * P)
    bkt = sl // (NB * P)
    off = sl % (NB * P)
    node = bkt * BUCK_N + off
    valid = (off < BUCK_N) & (node < N)
    sidx = np.full((NCORES, 2 * NB * P), ATRASH, np.int64)
    for c in range(NCORES):
        owned = valid & (node // SH == c)
        sidx[c] = np.where(owned, node - c * SH, ATRASH)
    sidx_p = np.stack([_pack_idx16(sidx[c]) for c in range(NCORES)])

    return chunks, gmax, cblk, s16, midx_p, aidx_p, dstf2, sidx_p


# ----------------------------------------------------------------------------
# device program
# ----------------------------------------------------------------------------

def _build_nc(chunks, gmax, cblk, s16):
    nc = bacc.Bacc("TRN2", target_bir_lowering=False, debug=False,
                   dynamic_dma_scratch_size=16384)

    t_xt = nc.dram_tensor("xt", [DIN, NXPAD], dt.bfloat16, kind="ExternalInput")
    t_wg = nc.dram_tensor("wg", [DIN, HF], dt.float32, kind="ExternalInput")
    t_asb = nc.dram_tensor("attsrc_b", [P, HF], dt.float32, kind="ExternalInput")
    t_adb = nc.dram_tensor("attdst_b", [P, HF], dt.float32, kind="ExternalInput")
    t_bg = nc.dram_tensor("biasg_b", [P, F], dt.float32, kind="ExternalInput")
    t_bp = nc.dram_tensor("bprior_b", [P, F], dt.float32, kind="ExternalInput")
    t_gate = nc.dram_tensor("gate_b", [P, 1], dt.float32, kind="ExternalInput")
    t_wp = nc.dram_tensor("wp16", [32, F], dt.bfloat16, kind="ExternalInput")
    t_kgt = nc.dram_tensor("kgt", [32, KGTC], dt.bfloat16, kind="ExternalInput")
    t_iota = nc.dram_tensor("iota_g", [P, gmax * P], dt.bfloat16,
                            kind="ExternalInput")
    t_midx = nc.dram_tensor("midx", [P, s16 // 16], dt.int16,
                            kind="ExternalInput")
    t_aidx = nc.dram_tensor("aidx", [P, s16 // 16], dt.int16,
                            kind="ExternalInput")
    t_dstf = nc.dram_tensor("dstf2", [P, cblk * 2], dt.bfloat16,
                            kind="ExternalInput")
    t_sidx = nc.dram_tensor("sidx", [P, (2 * NB * P) // 16], dt.int16,
                            kind="ExternalInput")
    t_out = nc.dram_tensor("out", [SH, F], dt.float32, kind="ExternalOutput")

    t_T = nc.dram_tensor("tbl", [2 * BROWS, ROWE], dt.bfloat16)
    t_A = nc.dram_tensor("adst", [AROWS, F], dt.float32)

    def v3(t, d1, s1, d2, s2):
        return bass.AP(t.tensor, t.offset, [t.ap[0], [s1, d1], [s2, d2]])

    def v4(t, d1, s1, d2, s2, d3, s3, off=0):
        return bass.AP(t.tensor, t.offset + off,
                       [t.ap[0], [s1, d1], [s2, d2], [s3, d3]])

    with tile.TileContext(nc) as tc:
        with ExitStack() as octx:
            cp = octx.enter_context(tc.tile_pool(name="const", bufs=1))

            iota_sb = cp.tile([P, gmax * P], dt.bfloat16)
            nc.sync.dma_start(iota_sb[:], t_iota[:])
            wp_sb = cp.tile([32, F], dt.bfloat16)
            nc.sync.dma_start(wp_sb[:], t_wp[:])
            kgt_sb = cp.tile([32, KGTC], dt.bfloat16)
            nc.sync.dma_start(kgt_sb[:], t_kgt[:])
            asb = cp.tile([P, HF], dt.float32)
            nc.sync.dma_start(asb[:], t_asb[:])
            adb = cp.tile([P, HF], dt.float32)
            nc.sync.dma_start(adb[:], t_adb[:])
            bg_sb = cp.tile([P, F], dt.float32)
            nc.sync.dma_start(bg_sb[:], t_bg[:])
            bp_sb = cp.tile([P, F], dt.float32)
            nc.sync.dma_start(bp_sb[:], t_bp[:])
            gate_sb = cp.tile([P, 1], dt.float32)
            nc.sync.dma_start(gate_sb[:], t_gate[:])
            g1m_sb = cp.tile([P, 1], dt.float32)
            nc.vector.tensor_scalar(g1m_sb[:], gate_sb[:], -1.0, 1.0,
                                    op0=Alu.mult, op1=Alu.add)
            midx_sb = cp.tile([P, s16 // 16], dt.int16)
            nc.sync.dma_start(midx_sb[:], t_midx[:])
            aidx_sb = cp.tile([P, s16 // 16], dt.int16)
            nc.sync.dma_start(aidx_sb[:], t_aidx[:])
            dstf_sb = cp.tile([P, cblk * 2], dt.bfloat16)
            nc.sync.dma_start(dstf_sb[:], t_dstf[:])
            sidx_sb = cp.tile([P, (2 * NB * P) // 16], dt.int16)
            nc.sync.dma_start(sidx_sb[:], t_sidx[:])

            # W_ext = [W_gat | wsrc(4) | wdst(4)] in (j, head) interleave;
            # wsrc_hd = sum_j W[:,(j,hd)]*att_src[(j,hd)]
            wext = cp.tile([P, HF + 8], dt.float32)
            nc.sync.dma_start(wext[:, 0:HF], t_wg[:])
            with tc.tile_pool(name="wtmp", bufs=2) as wtp:
                for j in range(HEADS):
                    for k, att in enumerate((asb, adb)):
                        tmp = wtp.tile([P, F], dt.float32, tag="wtmp")
                        nc.vector.tensor_tensor(
                            out=tmp[:],
                            in0=v3(bass.AP(wext[:].tensor, wext[:].offset + j,
                                           [wext[:].ap[0]]), F, 4, 1, 1),
                            in1=v3(bass.AP(att[:].tensor, att[:].offset + j,
                                           [att[:].ap[0]]), F, 4, 1, 1),
                            op=Alu.mult)
                        nc.vector.tensor_reduce(
                            out=wext[:, HF + 4 * k + j:HF + 4 * k + j + 1],
                            in_=tmp[:], axis=mybir.AxisListType.X, op=Alu.add)
            wext16 = cp.tile([P, HF + 8], dt.bfloat16)
            nc.vector.tensor_copy(out=wext16[:], in_=wext[:])

            # zero-fill the A-table (scatter_add target)
            zt = cp.tile([P, (AROWS // P) * F], dt.float32)
            nc.vector.memset(zt[:], 0.0)
            nc.sync.dma_start(
                t_A[:, :].rearrange("(p a) c -> p (a c)", p=P), zt[:])

            # ---------------- phase 1: build T and A ----------------
            with tc.tile_pool(name="p1x", bufs=3) as p1x, \
                 tc.tile_pool(name="p1r", bufs=3) as p1r, \
                 tc.tile_pool(name="p1s", bufs=2) as p1s, \
                 tc.tile_pool(name="p1ps", bufs=4, space="PSUM") as pp1, \
                 tc.tile_pool(name="p1ac", bufs=3) as p1a:
                xb = None
                rb = None
                ach_tile = None
                for bkt in range(2):
                    for bi in range(NB):
                        if bi % XB == 0:
                            colb = bkt * BUCK_N + bi * P
                            nx = min(XB, NB - bi) * P
                            xb = p1x.tile([P, XB * P], dt.bfloat16, tag="xb")
                            nc.sync.dma_start(xb[:, 0:nx],
                                              t_xt[:, colb:colb + nx])
                        h_ps = pp1.tile([P, HF + 8], dt.float32, space="PSUM",
                                        tag="hps")
                        nc.tensor.matmul(
                            h_ps[:],
                            lhsT=xb[:, (bi % XB) * P:(bi % XB + 1) * P],
                            rhs=wext16[:], start=True, stop=True)
                        # T-row staging (TB full blocks per write)
                        full = bi < NBF
                        if full:
                            ti = (bi % TB)
                            if ti == 0:
                                rb = p1r.tile([P, TB, ROWD], dt.bfloat16,
                                              tag="rb")
                            nc.vector.tensor_copy(out=rb[:, ti, 0:P],
                                                  in_=h_ps[:, 0:P])
                            nc.scalar.copy(rb[:, ti, P:ROWD],
                                           h_ps[:, P:ROWD])
                            if ti == TB - 1 or bi == NBF - 1:
                                nb = ti + 1
                                r0 = bkt * BROWS + (bi - ti) * P
                                dst_ap = t_T[r0:r0 + nb * P, 0:ROWD] \
                                    .rearrange("(b p) c -> p b c", p=P)
                                nc.sync.dma_start(dst_ap, rb[:, 0:nb, :])
                        else:
                            # partial tail block: write its rows directly
                            nr = BUCK_N - bi * P
                            rt = p1s.tile([P, ROWD], dt.bfloat16, tag="rt")
                            nc.vector.tensor_copy(out=rt[:, 0:P],
                                                  in_=h_ps[:, 0:P])
                            nc.scalar.copy(rt[:, P:ROWD], h_ps[:, P:ROWD])
                            nc.sync.dma_start(
                                t_T[bkt * BROWS + bi * P:
                                    bkt * BROWS + bi * P + nr, 0:ROWD],
                                rt[:nr, :])
                        ci = bi % ACH
                        if ci == 0:
                            ach_tile = p1a.tile([P, ACH, HEADS], dt.float32,
                                                tag="ach")
                        nc.scalar.copy(ach_tile[:, ci, :],
                                       h_ps[:, HF + 4:HF + 8])
                        if ci == ACH - 1 or bi == NB - 1:
                            nb = ci + 1
                            slot0 = (bkt * NB + bi - ci) * P
                            nc.gpsimd.dma_scatter_add(
                                t_A[:, 0:HEADS], ach_tile[:, 0:nb, :],
                                sidx_sb[:, slot0 // 16:(slot0 + nb * P) // 16],
                                nb * P, nb * P, HEADS, F)
                # sentinel rows (h=0, a_src=-1e30)
                sent = p1s.tile([1, ROWD], dt.bfloat16, tag="sent")
                nc.gpsimd.memset(sent[:], 0.0)
                nc.gpsimd.memset(sent[:, HF:ROWD], -1e30)
                nc.sync.dma_start(t_T[SENT:SENT + 1, 0:ROWD], sent[:])
                nc.sync.dma_start(t_T[BROWS + SENT:BROWS + SENT + 1, 0:ROWD],
                                  sent[:])

            # ---------------- phase 2: gather / scatter ----------------
            with tc.tile_pool(name="g1p", bufs=4) as g1p, \
                 tc.tile_pool(name="g2p", bufs=4) as g2p, \
                 tc.tile_pool(name="wk", bufs=4) as wk, \
                 tc.tile_pool(name="sp", bufs=4) as sp, \
                 tc.tile_pool(name="accp", bufs=4, space="PSUM") as accp, \
                 tc.tile_pool(name="prp", bufs=2, space="PSUM") as prp, \
                 tc.tile_pool(name="fin", bufs=4) as fin, \
                 tc.tile_pool(name="outp", bufs=2) as outp:

                acc_ps = None
                ob_tile = None
                for (wi, g, blk0, ic0, nn, tbase, first, last) in chunks:
                    if first:
                        acc_ps = accp.tile([P, HF + 4], dt.float32,
                                           space="PSUM", tag="acc")
                    g1 = g1p.tile([P, gmax, ROWE], dt.bfloat16, tag="g1")
                    nc.gpsimd.dma_gather(
                        g1[:, 0:g, :], t_T[tbase:tbase + BROWS, :],
                        midx_sb[:, ic0:ic0 + nn // 16], nn, nn, ROWE)
                    g2 = g2p.tile([P, gmax, F], dt.float32, tag="g2")
                    nc.gpsimd.dma_gather(
                        g2[:, 0:g, :], t_A[:, :],
                        aidx_sb[:, ic0:ic0 + nn // 16], nn, nn, F)

                    g1a = g1[:, 0:g, :]
                    # alpha = leaky(asrc + adst)
                    alpha = wk.tile([P, gmax * HEADS], dt.float32, tag="alpha")
                    nc.vector.tensor_tensor(
                        out=v3(alpha[:], g, 4, 4, 1),
                        in0=bass.AP(g1a.tensor, g1a.offset + HF,
                                    [g1a.ap[0], [ROWE, g], [1, 4]]),
                        in1=bass.AP(g2[:].tensor, g2[:].offset,
                                    [g2[:].ap[0], [F, g], [1, 4]]),
                        op=Alu.add)
                    nc.vector.scalar_tensor_tensor(
                        out=alpha[:, 0:g * 4], in0=alpha[:, 0:g * 4],
                        scalar=NEG_SLOPE, in1=alpha[:, 0:g * 4],
                        op0=Alu.mult, op1=Alu.max)
                    ex = wk.tile([P, gmax * HEADS], dt.bfloat16, tag="ex")
                    nc.scalar.activation(ex[:, 0:g * 4], alpha[:, 0:g * 4],
                                         Act.Exp)
                    # S[e, d] = (dstf == d)  [P, g, 128] bf16, 2x via dup'd
                    # dstf and (d1,d0) split so every last dim is packed
                    S = sp.tile([P, gmax * P], dt.bfloat16, tag="S")
                    nc.vector.tensor_tensor(
                        out=v4(S[:], g, P, F, 2, 2, 1),
                        in0=v4(dstf_sb[:, blk0 * 2:(blk0 + g) * 2],
                               g, 2, F, 0, 2, 1),
                        in1=v4(iota_sb[:, 0:g * P], g, 0, F, 2, 2, 1),
                        op=Alu.is_equal)
                    # rhs = [ex*h (256, (j,hd) order) | ex (4)]  [P, g, 260]
                    rhs = wk.tile([P, gmax * (HF + 4)], dt.bfloat16, tag="rhs")
                    nc.vector.tensor_tensor(
                        out=v4(rhs[:], g, HF + 4, F, 4, 4, 1),
                        in0=v4(g1a, g, ROWE, F, 4, 4, 1),
                        in1=v4(ex[:], g, 4, F, 0, 4, 1),
                        op=Alu.mult)
                    nc.scalar.copy(
                        bass.AP(rhs[:].tensor, rhs[:].offset + HF,
                                [rhs[:].ap[0], [HF + 4, g], [1, 4]]),
                        v3(ex[:], g, 4, 4, 1))
                    for k in range(g):
                        nc.tensor.matmul(
                            acc_ps[:],
                            lhsT=S[:, k * P:(k + 1) * P],
                            rhs=rhs[:, k * (HF + 4):(k + 1) * (HF + 4)],
                            start=(first and k == 0),
                            stop=(last and k == g - 1))

                    if last:
                        # ---- window epilogue (acc cols are (j, hd)) ----
                        nw = min(WIN, SH - wi * WIN)
                        den = fin.tile([P, HEADS], dt.float32, tag="den")
                        nc.vector.tensor_scalar(
                            den[:], acc_ps[:, HF:HF + 4], 1e-16, float(HEADS),
                            op0=Alu.add, op1=Alu.mult)
                        rec = fin.tile([P, HEADS], dt.float32, tag="rec")
                        nc.vector.reciprocal(rec[:], den[:])
                        prod = fin.tile([P, HF], dt.float32, tag="prod")
                        nc.vector.tensor_tensor(
                            out=v4(prod[:], F, 4, 1, 0, 4, 1),
                            in0=v4(acc_ps[:, 0:HF], F, 4, 1, 0, 4, 1),
                            in1=v4(rec[:], F, 0, 1, 0, 4, 1), op=Alu.mult)
                        oi = wi % OB
                        if oi == 0:
                            ob_tile = outp.tile([P, OB, F], dt.float32,
                                                tag="ot")
                        gat = fin.tile([P, F], dt.float32, tag="gat")
                        nc.vector.tensor_reduce(
                            out=gat[:],
                            in_=v3(prod[:], F, 4, HEADS, 1),
                            axis=mybir.AxisListType.X, op=Alu.add)
                        # prior = kgT_win.T @ W_prior
                        pr_ps = prp.tile([P, F], dt.float32, space="PSUM",
                                         tag="prps")
                        nc.tensor.matmul(pr_ps[:],
                                         lhsT=kgt_sb[:, wi * WIN:wi * WIN + P],
                                         rhs=wp_sb[:], start=True, stop=True)
                        # combine: (1-g)*(gat+bias) + g*(prior+bprior)
                        t1 = ob_tile[:, oi, :]
                        nc.vector.tensor_tensor(out=t1, in0=gat[:],
                                                in1=bg_sb[:], op=Alu.add)
                        nc.vector.tensor_scalar_mul(t1, t1, g1m_sb[:, 0:1])
                        t2 = fin.tile([P, F], dt.float32, tag="t2")
                        nc.vector.tensor_tensor(out=t2[:], in0=pr_ps[:],
                                                in1=bp_sb[:], op=Alu.add)
                        nc.vector.tensor_scalar_mul(t2[:], t2[:],
                                                    gate_sb[:, 0:1])
                        nc.vector.tensor_tensor(out=t1, in0=t1, in1=t2[:],
                                                op=Alu.add)
                        if oi == OB - 1 or wi == NWIN - 1:
                            nbw = oi + 1
                            w0 = wi - oi
                            nrows = min(OB * WIN, SH - w0 * WIN)
                            if nrows == nbw * WIN:
                                dst_ap = t_out[w0 * WIN:w0 * WIN + nrows, :] \
                                    .rearrange("(b p) c -> p b c", p=WIN)
                                nc.sync.dma_start(dst_ap,
                                                  ob_tile[:WIN, 0:nbw, :])
                            else:
                                # tail: full windows then the partial one
                                if nbw > 1:
                                    da = t_out[w0 * WIN:
                                               w0 * WIN + (nbw - 1) * WIN,
                                               :].rearrange(
                                        "(b p) c -> p b c", p=WIN)
                                    nc.sync.dma_start(
                                        da, ob_tile[:WIN, 0:nbw - 1, :])
                                nc.sync.dma_start(
                                    t_out[wi * WIN:wi * WIN + nw, :],
                                    ob_tile[:nw, nbw - 1, :])

    nc.compile()
    return nc


def _interleave_cols(a):
    """[..., HEADS*F] head-major -> (j, head) interleaved columns."""
    s = a.shape[:-1]
    return np.ascontiguousarray(
        a.reshape(s + (HEADS, F)).swapaxes(-1, -2).reshape(s + (HEADS * F,)))


def kernel(**inputs):
    x = np.asarray(inputs["x"], np.float32)
    edge_index = np.asarray(inputs["edge_index"])
    kg = np.asarray(inputs["kg_onehot"], np.float32)
    wg = np.ascontiguousarray(np.asarray(inputs["W_gat"], np.float32))
    att_src = np.asarray(inputs["att_src"], np.float32)
    att_dst = np.asarray(inputs["att_dst"], np.float32)
    bias_gat = np.asarray(inputs["bias_gat"], np.float32)
    wp = np.asarray(inputs["W_prior"], np.float32)
    b_prior = np.asarray(inputs["b_prior"], np.float32)
    gate = np.asarray(inputs["gate"], np.float32)

    chunks, gmax, cblk, s16, midx_p, aidx_p, dstf2, sidx_p = \
        _prep_edges(edge_index)

    xt = np.zeros((DIN, NXPAD), BF16)
    xt[:, 0:N] = np.ascontiguousarray(x.T).astype(BF16)
    iota_g = np.broadcast_to(
        np.tile(np.arange(P, dtype=np.float32), gmax)[None, :],
        (P, gmax * P)).astype(BF16)

    wg_i = _interleave_cols(wg)                      # [128, 256] (j, hd)
    att_src_i = _interleave_cols(att_src.reshape(1, HF))
    att_dst_i = _interleave_cols(att_dst.reshape(1, HF))

    shared = {
        "xt": xt, "wg": wg_i,
        "wp16": wp.astype(BF16),
        "attsrc_b": np.broadcast_to(att_src_i, (P, HF)).copy(),
        "attdst_b": np.broadcast_to(att_dst_i, (P, HF)).copy(),
        "biasg_b": np.broadcast_to(bias_gat.reshape(1, F), (P, F)).copy(),
        "bprior_b": np.broadcast_to(b_prior.reshape(1, F), (P, F)).copy(),
        "gate_b": np.broadcast_to(gate.reshape(1, 1), (P, 1)).copy(),
        "iota_g": np.ascontiguousarray(iota_g),
    }
    in_maps = []
    for c in range(NCORES):
        m = dict(shared)
        m["midx"] = midx_p[c]
        m["aidx"] = aidx_p[c]
        m["dstf2"] = dstf2[c]
        m["sidx"] = sidx_p[c]
        kgt = np.zeros((32, KGTC), BF16)
        kgt[:, 0:SH] = kg[c * SH:(c + 1) * SH].T.astype(BF16)
        m["kgt"] = kgt
        in_maps.append(m)

    nc = _build_nc(chunks, gmax, cblk, s16)
    res = run_bass_kernel_spmd(nc, in_maps, core_ids=list(range(NCORES)))
    out = np.concatenate([res.results[c]["out"] for c in range(NCORES)],
                         axis=0)
    return out.astype(np.float32)


# revision 23
# speedup vs baseline: 1.4927x; 1.0282x over previous
"""Trainium2 Bass kernel for ClimateConditionedGAT (GATConv + one-hot prior gate).

Strategy (8 NeuronCores, SPMD single NEFF):
  - Nodes sharded by destination: core c owns dst nodes [c*6250, (c+1)*6250).
  - Head-interleaved feature layout: h columns are (j, head) pairs, i.e.
    col 4*j+hd = h[hd*64+j]. This makes the per-edge ex broadcast and the
    one-hot compare packed in their last AP dim, unlocking the DVE 2x_1p
    mode (2-byte dtypes).
  - Phase 1 (replicated on every core): h = x @ W_ext from a host-transposed
    bf16 xT (no PE transpose). Per 128-node block the [128,264] f32 PSUM
    result yields:
      * T-row (260 bf16 = 520B payload in a 768B-stride row): [h(256)|a_src(4)]
        written in batches of 15 blocks per DMA; two 25001-row source buckets
        (dma_gather indices are int16), each with a sentinel row
        (a_src = -1e30 => exp -> 0).
      * a_dst (4 f32) staged and dma_scatter_add'ed into a per-core LOCAL
        A-table [6272, 64] f32 via host-provided indices (own dst rows at
        0..6249, everything else onto trash row 6250), so phase 2 can fetch
        a_dst with a direct int16 gather.
  - Phase 2: edges grouped by (dst-window of 128, src bucket), padded to the
    max count over cores (16-granular; shared instruction stream). Per group:
    one 768B-row dma_gather of T by src and one 256B-row dma_gather of A by
    local dst; ex = exp(leaky(asrc+adst)); S[e,d] = (dstf==d) (bf16, 2x) and
    rhs = [ex*h | ex] (bf16, 2x) feed PE bf16 matmuls accumulating
    [sum ex*h | sum ex] into a per-window PSUM tile; the window epilogue
    normalizes, averages heads, adds bias, computes prior = kg @ W_prior
    (host-transposed kgT, single preload) and gates the two; outputs are
    staged and written 8 windows per DMA.
All float math on device; host does integer edge partitioning/packing and
dtype/layout conversion of inputs only.
"""
import sys
import numpy as np

if "/opt/trn_rl_repo" not in sys.path:
    sys.path.insert(0, "/opt/trn_rl_repo")

from contextlib import ExitStack

import ml_dtypes

import concourse.bass as bass
import concourse.bacc as bacc
import concourse.mybir as mybir
import concourse.tile as tile
from concourse.bass_utils import run_bass_kernel_spmd

BF16 = ml_dtypes.bfloat16

P = 128
N = 50000
DIN = 128
HEADS = 4
F = 64
HF = HEADS * F          # 256
NCORES = 8
SH = N // NCORES        # 6250 dst nodes per core
ROWE = 384              # T row stride in bf16 elems (768 B)
ROWD = 260              # written part of a T row (520 B): h(256) + asrc(4)
BUCK_N = 25000          # real nodes per source bucket
BROWS = BUCK_N + 1      # bucket rows incl sentinel
SENT = BUCK_N           # bucket-local sentinel index
NB = (BUCK_N + P - 1) // P      # 196 phase-1 blocks per bucket
NBF = BUCK_N // P               # 195 full blocks
WIN = 96                # dst window size (<=128; sized so one (w,b) group
                        # usually fits a single 1024-index dma_gather)
NWIN = (SH + WIN - 1) // WIN    # 66 windows per core
AROWS = 6272            # A-table rows; trash row = SH
KGTC = NWIN * WIN + P   # kgt padded cols (window slices are 128 wide)
ATRASH = SH
NXPAD = 2 * NB * P      # padded xT columns (50176)
ACH = 32                # blocks of a_dst staged per scatter
TB = 15                 # full blocks of T rows staged per write
XB = 8                  # blocks of x loaded per DMA
OB = 8                  # windows of output staged per write
NEG_SLOPE = 0.2
dt = mybir.dt
Alu = mybir.AluOpType
Act = mybir.ActivationFunctionType


# ----------------------------------------------------------------------------
# host-side edge preparation (integer/layout work only)
# ----------------------------------------------------------------------------

def _pack_idx16(idx):
    """Pack int array -> [128, ceil(n/16)] int16 (wrap 16, replicate x8)."""
    n = len(idx)
    cols = (n + 15) // 16
    a = np.zeros((16, cols), np.int16)
    a[np.arange(n) % 16, np.arange(n) // 16] = idx.astype(np.int16)
    return np.tile(a, (8, 1))


def _prep_edges(edge_index):
    src = np.asarray(edge_index[0], dtype=np.int64)
    dst = np.asarray(edge_index[1], dtype=np.int64)
    loops = np.arange(N, dtype=np.int64)
    src = np.concatenate([src, loops])
    dst = np.concatenate([dst, loops])

    core = dst // SH
    dl = dst % SH
    w = dl // WIN
    b = (src >= BUCK_N).astype(np.int64)

    gid = (core * NWIN + w) * 2 + b
    ngroups = NCORES * NWIN * 2
    counts = np.bincount(gid, minlength=ngroups).reshape(NCORES, NWIN, 2)

    # uniform counts across cores (shared NEFF), 16-granular
    n_wb = counts.max(axis=0)                        # [NWIN, 2]
    n16 = np.maximum(((n_wb + 15) // 16) * 16, 16)   # [NWIN, 2]
    g_wb = (n16 + P - 1) // P                        # blocks per group
    gmax = int(g_wb.max())
    s16 = int(n16.sum())                             # total gathered slots
    cblk = int(g_wb.sum())                           # total blocks

    # chunk list: (w, g, blk0, ic0, nidx, tbase, first, last) in emission
    # order; groups larger than GSPL blocks are split so one dma_gather never
    # exceeds GSPL*128 indices (SWDGE descriptor ring capacity)
    GSPL = 8
    chunks = []
    ic = 0
    blk0 = 0
    for wi in range(NWIN):
        for bi in range(2):
            rem = int(g_wb[wi, bi])
            nrem = int(n16[wi, bi])
            while rem > 0:
                gg = min(rem, GSPL)
                nn = min(nrem, gg * P)
                first = bi == 0 and rem == int(g_wb[wi, 0])
                last = bi == 1 and rem == gg
                chunks.append((wi, gg, blk0, ic // 16, nn,
                               bi * BROWS, first, last))
                ic += nn
                blk0 += gg
                rem -= gg
                nrem -= nn
    gmax = min(gmax, GSPL)

    order = np.argsort(gid, kind="stable")
    src_s, dst_s = src[order], dst[order]
    grp_starts = np.zeros(ngroups + 1, np.int64)
    np.cumsum(np.bincount(gid[order], minlength=ngroups), out=grp_starts[1:])

    s_off = np.zeros((NWIN, 2), np.int64)
    b_off = np.zeros((NWIN, 2), np.int64)
    o = 0
    ob = 0
    for wi in range(NWIN):
        for bi in range(2):
            s_off[wi, bi] = o
            b_off[wi, bi] = ob
            o += int(n16[wi, bi])
            ob += int(g_wb[wi, bi])

    midx = np.full((NCORES, s16), SENT, np.int64)    # bucket-local src row
    aidx = np.full((NCORES, s16), ATRASH, np.int64)  # core-local dst row
    dstf = np.full((NCORES, cblk, P), -1.0, np.float32)  # dst local to window

    for c in range(NCORES):
        for wi in range(NWIN):
            for bi in range(2):
                gsl = (c * NWIN + wi) * 2 + bi
                a0, a1 = grp_starts[gsl], grp_starts[gsl + 1]
                n = int(a1 - a0)
                if n == 0:
                    continue
                o = int(s_off[wi, bi])
                es, ed = src_s[a0:a1], dst_s[a0:a1]
                edl = ed % SH
                midx[c, o:o + n] = es - bi * BUCK_N
                aidx[c, o:o + n] = edl
                sl = np.arange(n)
                blk = int(b_off[wi, bi]) + sl // P
                dstf[c, blk, sl % P] = (edl - wi * WIN).astype(np.float32)

    midx_p = np.stack([_pack_idx16(midx[c]) for c in range(NCORES)])
    aidx_p = np.stack([_pack_idx16(aidx[c]) for c in range(NCORES)])
    # dstf2: each value duplicated along a new inner dim of 2 (packed last
    # dim for the DVE 2x one-hot compare): [P, cblk*2]
    dstf2 = np.repeat(dstf.transpose(0, 2, 1), 2, axis=2).astype(BF16)

    # phase-1 a_dst scatter indices: slot = (bkt*NB + bi)*128 + p
    sl = np.arange(2 * NB * P)
    bkt = sl // (NB * P)
    off = sl % (NB * P)
    node = bkt * BUCK_N + off
    valid = (off < BUCK_N) & (node < N)
    sidx = np.full((NCORES, 2 * NB * P), ATRASH, np.int64)
    for c in range(NCORES):
        owned = valid & (node // SH == c)
        sidx[c] = np.where(owned, node - c * SH, ATRASH)
    sidx_p = np.stack([_pack_idx16(sidx[c]) for c in range(NCORES)])

    return chunks, gmax, cblk, s16, midx_p, aidx_p, dstf2, sidx_p


# ----------------------------------------------------------------------------
# device program
# ----------------------------------------------------------------------------

def _build_nc(chunks, gmax, cblk, s16):
    nc = bacc.Bacc("TRN2", target_bir_lowering=False, debug=False,
                   dynamic_dma_scratch_size=16384)

    t_xt = nc.dram_tensor("xt", [DIN, NXPAD], dt.bfloat16, kind="ExternalInput")
    t_wg = nc.dram_tensor("wg", [DIN, HF], dt.float32, kind="ExternalInput")
    t_asb = nc.dram_tensor("attsrc_b", [P, HF], dt.float32, kind="ExternalInput")
    t_adb = nc.dram_tensor("attdst_b", [P, HF], dt.float32, kind="ExternalInput")
    t_bg = nc.dram_tensor("biasg_b", [P, F], dt.float32, kind="ExternalInput")
    t_bp = nc.dram_tensor("bprior_b", [P, F], dt.float32, kind="ExternalInput")
    t_gate = nc.dram_tensor("gate_b", [P, 1], dt.float32, kind="ExternalInput")
    t_wp = nc.dram_tensor("wp16", [32, F], dt.bfloat16, kind="ExternalInput")
    t_kgt = nc.dram_tensor("kgt", [32, KGTC], dt.bfloat16, kind="ExternalInput")
    t_iota = nc.dram_tensor("iota_g", [P, gmax * P], dt.bfloat16,
                            kind="ExternalInput")
    t_midx = nc.dram_tensor("midx", [P, s16 // 16], dt.int16,
                            kind="ExternalInput")
    t_aidx = nc.dram_tensor("aidx", [P, s16 // 16], dt.int16,
                            kind="ExternalInput")
    t_dstf = nc.dram_tensor("dstf2", [P, cblk * 2], dt.bfloat16,
                            kind="ExternalInput")
    t_sidx = nc.dram_tensor("sidx", [P, (2 * NB * P) // 16], dt.int16,
                            kind="ExternalInput")
    t_out = nc.dram_tensor("out", [SH, F], dt.float32, kind="ExternalOutput")

    t_T = nc.dram_tensor("tbl", [2 * BROWS, ROWE], dt.bfloat16)
    t_A = nc.dram_tensor("adst", [AROWS, F], dt.float32)

    def v3(t, d1, s1, d2, s2):
        return bass.AP(t.tensor, t.offset, [t.ap[0], [s1, d1], [s2, d2]])

    def v4(t, d1, s1, d2, s2, d3, s3, off=0):
        return bass.AP(t.tensor, t.offset + off,
                       [t.ap[0], [s1, d1], [s2, d2], [s3, d3]])

    with tile.TileContext(nc) as tc:
        with ExitStack() as octx:
            cp = octx.enter_context(tc.tile_pool(name="const", bufs=1))

            iota_sb = cp.tile([P, gmax * P], dt.bfloat16)
            nc.sync.dma_start(iota_sb[:], t_iota[:])
            wp_sb = cp.tile([32, F], dt.bfloat16)
            nc.sync.dma_start(wp_sb[:], t_wp[:])
            kgt_sb = cp.tile([32, KGTC], dt.bfloat16)
            nc.sync.dma_start(kgt_sb[:], t_kgt[:])
            asb = cp.tile([P, HF], dt.float32)
            nc.sync.dma_start(asb[:], t_asb[:])
            adb = cp.tile([P, HF], dt.float32)
            nc.sync.dma_start(adb[:], t_adb[:])
            bg_sb = cp.tile([P, F], dt.float32)
            nc.sync.dma_start(bg_sb[:], t_bg[:])
            bp_sb = cp.tile([P, F], dt.float32)
            nc.sync.dma_start(bp_sb[:], t_bp[:])
            gate_sb = cp.tile([P, 1], dt.float32)
            nc.sync.dma_start(gate_sb[:], t_gate[:])
            g1m_sb = cp.tile([P, 1], dt.float32)
            nc.vector.tensor_scalar(g1m_sb[:], gate_sb[:], -1.0, 1.0,
                                    op0=Alu.mult, op1=Alu.add)
            cb_sb = cp.tile([P, F], dt.float32)
            nc.vector.tensor_scalar_mul(cb_sb[:], bg_sb[:], g1m_sb[:, 0:1])
            cb2 = cp.tile([P, F], dt.float32)
            nc.vector.tensor_scalar_mul(cb2[:], bp_sb[:], gate_sb[:, 0:1])
            nc.vector.tensor_tensor(out=cb_sb[:], in0=cb_sb[:], in1=cb2[:],
                                    op=Alu.add)
            midx_sb = cp.tile([P, s16 // 16], dt.int16)
            nc.sync.dma_start(midx_sb[:], t_midx[:])
            aidx_sb = cp.tile([P, s16 // 16], dt.int16)
            nc.sync.dma_start(aidx_sb[:], t_aidx[:])
            dstf_sb = cp.tile([P, cblk * 2], dt.bfloat16)
            nc.sync.dma_start(dstf_sb[:], t_dstf[:])
            sidx_sb = cp.tile([P, (2 * NB * P) // 16], dt.int16)
            nc.sync.dma_start(sidx_sb[:], t_sidx[:])

            # W_ext = [W_gat | wsrc(4) | wdst(4)] in (j, head) interleave;
            # wsrc_hd = sum_j W[:,(j,hd)]*att_src[(j,hd)]
            wext = cp.tile([P, HF + 8], dt.float32)
            nc.sync.dma_start(wext[:, 0:HF], t_wg[:])
            with tc.tile_pool(name="wtmp", bufs=2) as wtp:
                for j in range(HEADS):
                    for k, att in enumerate((asb, adb)):
                        tmp = wtp.tile([P, F], dt.float32, tag="wtmp")
                        nc.vector.tensor_tensor(
                            out=tmp[:],
                            in0=v3(bass.AP(wext[:].tensor, wext[:].offset + j,
                                           [wext[:].ap[0]]), F, 4, 1, 1),
                            in1=v3(bass.AP(att[:].tensor, att[:].offset + j,
                                           [att[:].ap[0]]), F, 4, 1, 1),
                            op=Alu.mult)
                        nc.vector.tensor_reduce(
                            out=wext[:, HF + 4 * k + j:HF + 4 * k + j + 1],
                            in_=tmp[:], axis=mybir.AxisListType.X, op=Alu.add)
            wext16 = cp.tile([P, HF + 8], dt.bfloat16)
            nc.vector.tensor_copy(out=wext16[:], in_=wext[:])

            # zero-fill the A-table (scatter_add target)
            zt = cp.tile([P, (AROWS // P) * F], dt.float32)
            nc.vector.memset(zt[:], 0.0)
            nc.sync.dma_start(
                t_A[:, :].rearrange("(p a) c -> p (a c)", p=P), zt[:])

            # ---------------- phase 1: build T and A ----------------
            with tc.tile_pool(name="p1x", bufs=3) as p1x, \
                 tc.tile_pool(name="p1r", bufs=3) as p1r, \
                 tc.tile_pool(name="p1s", bufs=2) as p1s, \
                 tc.tile_pool(name="p1ps", bufs=4, space="PSUM") as pp1, \
                 tc.tile_pool(name="p1ac", bufs=3) as p1a:
                xb = None
                rb = None
                ach_tile = None
                for bkt in range(2):
                    for bi in range(NB):
                        if bi % XB == 0:
                            colb = bkt * BUCK_N + bi * P
                            nx = min(XB, NB - bi) * P
                            xb = p1x.tile([P, XB * P], dt.bfloat16, tag="xb")
                            nc.sync.dma_start(xb[:, 0:nx],
                                              t_xt[:, colb:colb + nx])
                        h_ps = pp1.tile([P, HF + 8], dt.float32, space="PSUM",
                                        tag="hps")
                        nc.tensor.matmul(
                            h_ps[:],
                            lhsT=xb[:, (bi % XB) * P:(bi % XB + 1) * P],
                            rhs=wext16[:], start=True, stop=True)
                        # T-row staging (TB full blocks per write)
                        full = bi < NBF
                        if full:
                            ti = (bi % TB)
                            if ti == 0:
                                rb = p1r.tile([P, TB, ROWD], dt.bfloat16,
                                              tag="rb")
                            nc.vector.tensor_copy(out=rb[:, ti, 0:P],
                                                  in_=h_ps[:, 0:P])
                            nc.scalar.copy(rb[:, ti, P:ROWD],
                                           h_ps[:, P:ROWD])
                            if ti == TB - 1 or bi == NBF - 1:
                                nb = ti + 1
                                r0 = bkt * BROWS + (bi - ti) * P
                                dst_ap = t_T[r0:r0 + nb * P, 0:ROWD] \
                                    .rearrange("(b p) c -> p b c", p=P)
                                nc.sync.dma_start(dst_ap, rb[:, 0:nb, :])
                        else:
                            # partial tail block: write its rows directly
                            nr = BUCK_N - bi * P
                            rt = p1s.tile([P, ROWD], dt.bfloat16, tag="rt")
                            nc.vector.tensor_copy(out=rt[:, 0:P],
                                                  in_=h_ps[:, 0:P])
                            nc.scalar.copy(rt[:, P:ROWD], h_ps[:, P:ROWD])
                            nc.sync.dma_start(
                                t_T[bkt * BROWS + bi * P:
                                    bkt * BROWS + bi * P + nr, 0:ROWD],
                                rt[:nr, :])
                        ci = bi % ACH
                        if ci == 0:
                            ach_tile = p1a.tile([P, ACH, HEADS], dt.float32,
                                                tag="ach")
                        nc.scalar.copy(ach_tile[:, ci, :],
                                       h_ps[:, HF + 4:HF + 8])
                        if ci == ACH - 1 or bi == NB - 1:
                            nb = ci + 1
                            slot0 = (bkt * NB + bi - ci) * P
                            nc.gpsimd.dma_scatter_add(
                                t_A[:, 0:HEADS], ach_tile[:, 0:nb, :],
                                sidx_sb[:, slot0 // 16:(slot0 + nb * P) // 16],
                                nb * P, nb * P, HEADS, F)
                # sentinel rows (h=0, a_src=-1e30)
                sent = p1s.tile([1, ROWD], dt.bfloat16, tag="sent")
                nc.gpsimd.memset(sent[:], 0.0)
                nc.gpsimd.memset(sent[:, HF:ROWD], -1e30)
                nc.sync.dma_start(t_T[SENT:SENT + 1, 0:ROWD], sent[:])
                nc.sync.dma_start(t_T[BROWS + SENT:BROWS + SENT + 1, 0:ROWD],
                                  sent[:])

            # ---------------- phase 2: gather / scatter ----------------
            with tc.tile_pool(name="g1p", bufs=4) as g1p, \
                 tc.tile_pool(name="g2p", bufs=4) as g2p, \
                 tc.tile_pool(name="wk", bufs=4) as wk, \
                 tc.tile_pool(name="sp", bufs=4) as sp, \
                 tc.tile_pool(name="accp", bufs=4, space="PSUM") as accp, \
                 tc.tile_pool(name="prp", bufs=2, space="PSUM") as prp, \
                 tc.tile_pool(name="fin", bufs=4) as fin, \
                 tc.tile_pool(name="outp", bufs=2) as outp:

                # one-time init: tail slots of partially-gathered blocks
                # are read (and zeroed via S) but must hold finite data
                for _ in range(4):
                    gw1 = g1p.tile([P, gmax, ROWE], dt.bfloat16, tag="g1")
                    nc.vector.memset(gw1[:], 0.0)
                    gw2 = g2p.tile([P, gmax, F], dt.float32, tag="g2")
                    nc.vector.memset(gw2[:], 0.0)

                acc_ps = None
                ob_tile = None
                for (wi, g, blk0, ic0, nn, tbase, first, last) in chunks:
                    if first:
                        acc_ps = accp.tile([P, HF + 4], dt.float32,
                                           space="PSUM", tag="acc")
                    g1 = g1p.tile([P, gmax, ROWE], dt.bfloat16, tag="g1")
                    nc.gpsimd.dma_gather(
                        g1[:, 0:g, :], t_T[tbase:tbase + BROWS, :],
                        midx_sb[:, ic0:ic0 + nn // 16], nn, nn, ROWE)
                    g2 = g2p.tile([P, gmax, F], dt.float32, tag="g2")
                    nc.gpsimd.dma_gather(
                        g2[:, 0:g, :], t_A[:, :],
                        aidx_sb[:, ic0:ic0 + nn // 16], nn, nn, F)

                    g1a = g1[:, 0:g, :]
                    # alpha = leaky(asrc + adst)
                    alpha = wk.tile([P, gmax * HEADS], dt.float32, tag="alpha")
                    nc.vector.tensor_tensor(
                        out=v3(alpha[:], g, 4, 4, 1),
                        in0=bass.AP(g1a.tensor, g1a.offset + HF,
                                    [g1a.ap[0], [ROWE, g], [1, 4]]),
                        in1=bass.AP(g2[:].tensor, g2[:].offset,
                                    [g2[:].ap[0], [F, g], [1, 4]]),
                        op=Alu.add)
                    nc.vector.scalar_tensor_tensor(
                        out=alpha[:, 0:g * 4], in0=alpha[:, 0:g * 4],
                        scalar=NEG_SLOPE, in1=alpha[:, 0:g * 4],
                        op0=Alu.mult, op1=Alu.max)
                    ex = wk.tile([P, gmax * HEADS], dt.bfloat16, tag="ex")
                    nc.scalar.activation(ex[:, 0:g * 4], alpha[:, 0:g * 4],
                                         Act.Exp)
                    # S[e, d] = (dstf == d)  [P, g, 128] bf16, 2x via dup'd
                    # dstf and (d1,d0) split so every last dim is packed
                    S = sp.tile([P, gmax * P], dt.bfloat16, tag="S")
                    nc.vector.tensor_tensor(
                        out=v4(S[:], g, P, F, 2, 2, 1),
                        in0=v4(dstf_sb[:, blk0 * 2:(blk0 + g) * 2],
                               g, 2, F, 0, 2, 1),
                        in1=v4(iota_sb[:, 0:g * P], g, 0, F, 2, 2, 1),
                        op=Alu.is_equal)
                    # rhs = [ex*h (256, (j,hd) order) | ex (4)]  [P, g, 260]
                    rhs = wk.tile([P, gmax * (HF + 4)], dt.bfloat16, tag="rhs")
                    nc.vector.tensor_tensor(
                        out=v4(rhs[:], g, HF + 4, F, 4, 4, 1),
                        in0=v4(g1a, g, ROWE, F, 4, 4, 1),
                        in1=v4(ex[:], g, 4, F, 0, 4, 1),
                        op=Alu.mult)
                    nc.scalar.copy(
                        bass.AP(rhs[:].tensor, rhs[:].offset + HF,
                                [rhs[:].ap[0], [HF + 4, g], [1, 4]]),
                        v3(ex[:], g, 4, 4, 1))
                    for k in range(g):
                        nc.tensor.matmul(
                            acc_ps[:],
                            lhsT=S[:, k * P:(k + 1) * P],
                            rhs=rhs[:, k * (HF + 4):(k + 1) * (HF + 4)],
                            start=(first and k == 0),
                            stop=(last and k == g - 1))

                    if last:
                        # ---- window epilogue (acc cols are (j, hd)) ----
                        nw = min(WIN, SH - wi * WIN)
                        den = fin.tile([P, HEADS], dt.float32, tag="den")
                        nc.vector.tensor_scalar(
                            den[:], acc_ps[:, HF:HF + 4], 1e-16, float(HEADS),
                            op0=Alu.add, op1=Alu.mult)
                        rec = fin.tile([P, HEADS], dt.float32, tag="rec")
                        nc.vector.reciprocal(rec[:], den[:])
                        prod = fin.tile([P, HF], dt.float32, tag="prod")
                        nc.vector.tensor_tensor(
                            out=v4(prod[:], F, 4, 1, 0, 4, 1),
                            in0=v4(acc_ps[:, 0:HF], F, 4, 1, 0, 4, 1),
                            in1=v4(rec[:], F, 0, 1, 0, 4, 1), op=Alu.mult)
                        oi = wi % OB
                        if oi == 0:
                            ob_tile = outp.tile([P, OB, F], dt.float32,
                                                tag="ot")
                        gat = fin.tile([P, F], dt.float32, tag="gat")
                        nc.vector.tensor_reduce(
                            out=gat[:],
                            in_=v3(prod[:], F, 4, HEADS, 1),
                            axis=mybir.AxisListType.X, op=Alu.add)
                        # prior = kgT_win.T @ W_prior
                        pr_ps = prp.tile([P, F], dt.float32, space="PSUM",
                                         tag="prps")
                        nc.tensor.matmul(pr_ps[:],
                                         lhsT=kgt_sb[:, wi * WIN:wi * WIN + P],
                                         rhs=wp_sb[:], start=True, stop=True)
                        # combine: (1-g)*(gat+bias) + g*(prior+bprior)
                        t1 = ob_tile[:, oi, :]
                        nc.vector.tensor_scalar_mul(t1, gat[:], g1m_sb[:, 0:1])
                        t2 = fin.tile([P, F], dt.float32, tag="t2")
                        nc.vector.tensor_scalar_mul(t2[:], pr_ps[:],
                                                    gate_sb[:, 0:1])
                        nc.vector.tensor_tensor(out=t1, in0=t1, in1=t2[:],
                                                op=Alu.add)
                        nc.vector.tensor_tensor(out=t1, in0=t1, in1=cb_sb[:],
                                                op=Alu.add)
                        if oi == OB - 1 or wi == NWIN - 1:
                            nbw = oi + 1
                            w0 = wi - oi
                            nrows = min(OB * WIN, SH - w0 * WIN)
                            if nrows == nbw * WIN:
                                dst_ap = t_out[w0 * WIN:w0 * WIN + nrows, :] \
                                    .rearrange("(b p) c -> p b c", p=WIN)
                                nc.sync.dma_start(dst_ap,
                                                  ob_tile[:WIN, 0:nbw, :])
                            else:
                                # tail: full windows then the partial one
                                if nbw > 1:
                                    da = t_out[w0 * WIN:
                                               w0 * WIN + (nbw - 1) * WIN,
                                               :].rearrange(
                                        "(b p) c -> p b c", p=WIN)
                                    nc.sync.dma_start(
                                        da, ob_tile[:WIN, 0:nbw - 1, :])
                                nc.sync.dma_start(
                                    t_out[wi * WIN:wi * WIN + nw, :],
                                    ob_tile[:nw, nbw - 1, :])

    nc.compile()
    return nc


def _interleave_cols(a):
    """[..., HEADS*F] head-major -> (j, head) interleaved columns."""
    s = a.shape[:-1]
    return np.ascontiguousarray(
        a.reshape(s + (HEADS, F)).swapaxes(-1, -2).reshape(s + (HEADS * F,)))


def kernel(**inputs):
    x = np.asarray(inputs["x"], np.float32)
    edge_index = np.asarray(inputs["edge_index"])
    kg = np.asarray(inputs["kg_onehot"], np.float32)
    wg = np.ascontiguousarray(np.asarray(inputs["W_gat"], np.float32))
    att_src = np.asarray(inputs["att_src"], np.float32)
    att_dst = np.asarray(inputs["att_dst"], np.float32)
    bias_gat = np.asarray(inputs["bias_gat"], np.float32)
    wp = np.asarray(inputs["W_prior"], np.float32)
    b_prior = np.asarray(inputs["b_prior"], np.float32)
    gate = np.asarray(inputs["gate"], np.float32)

    chunks, gmax, cblk, s16, midx_p, aidx_p, dstf2, sidx_p = \
        _prep_edges(edge_index)

    xt = np.zeros((DIN, NXPAD), BF16)
    xt[:, 0:N] = np.ascontiguousarray(x.T).astype(BF16)
    iota_g = np.broadcast_to(
        np.tile(np.arange(P, dtype=np.float32), gmax)[None, :],
        (P, gmax * P)).astype(BF16)

    wg_i = _interleave_cols(wg)                      # [128, 256] (j, hd)
    att_src_i = _interleave_cols(att_src.reshape(1, HF))
    att_dst_i = _interleave_cols(att_dst.reshape(1, HF))

    shared = {
        "xt": xt, "wg": wg_i,
        "wp16": wp.astype(BF16),
        "attsrc_b": np.broadcast_to(att_src_i, (P, HF)).copy(),
        "attdst_b": np.broadcast_to(att_dst_i, (P, HF)).copy(),
        "biasg_b": np.broadcast_to(bias_gat.reshape(1, F), (P, F)).copy(),
        "bprior_b": np.broadcast_to(b_prior.reshape(1, F), (P, F)).copy(),
        "gate_b": np.broadcast_to(gate.reshape(1, 1), (P, 1)).copy(),
        "iota_g": np.ascontiguousarray(iota_g),
    }
    in_maps = []
    for c in range(NCORES):
        m = dict(shared)
        m["midx"] = midx_p[c]
        m["aidx"] = aidx_p[c]
        m["dstf2"] = dstf2[c]
        m["sidx"] = sidx_p[c]
        kgt = np.zeros((32, KGTC), BF16)
        kgt[:, 0:SH] = kg[c * SH:(c + 1) * SH].T.astype(BF16)
        m["kgt"] = kgt
        in_maps.append(m)

    nc = _build_nc(chunks, gmax, cblk, s16)
    res = run_bass_kernel_spmd(nc, in_maps, core_ids=list(range(NCORES)))
    out = np.concatenate([res.results[c]["out"] for c in range(NCORES)],
                         axis=0)
    return out.astype(np.float32)


# revision 26
# speedup vs baseline: 1.5895x; 1.0648x over previous
"""Trainium2 Bass kernel for ClimateConditionedGAT (GATConv + one-hot prior gate).

Strategy (8 NeuronCores, SPMD single NEFF):
  - Nodes sharded by destination: core c owns dst nodes [c*6250, (c+1)*6250).
  - Head-interleaved feature layout: h columns are (j, head) pairs, i.e.
    col 4*j+hd = h[hd*64+j]. This makes the per-edge ex broadcast and the
    one-hot compare packed in their last AP dim, unlocking the DVE 2x_1p
    mode (2-byte dtypes).
  - Phase 1 (replicated on every core): h = x @ W_ext from a host-transposed
    bf16 xT (no PE transpose). Per 128-node block the [128,264] f32 PSUM
    result yields:
      * T-row (260 bf16 = 520B payload in a 768B-stride row): [h(256)|a_src(4)]
        written in batches of 15 blocks per DMA; two 25001-row source buckets
        (dma_gather indices are int16), each with a sentinel row
        (a_src = -1e30 => exp -> 0).
      * a_dst (4 f32) staged and dma_scatter_add'ed into a per-core LOCAL
        A-table [6272, 64] f32 via host-provided indices (own dst rows at
        0..6249, everything else onto trash row 6250), so phase 2 can fetch
        a_dst with a direct int16 gather.
  - Phase 2: edges grouped by (dst-window of 128, src bucket), padded to the
    max count over cores (16-granular; shared instruction stream). Per group:
    one 768B-row dma_gather of T by src and one 256B-row dma_gather of A by
    local dst; ex = exp(leaky(asrc+adst)); S[e,d] = (dstf==d) (bf16, 2x) and
    rhs = [ex*h | ex] (bf16, 2x) feed PE bf16 matmuls accumulating
    [sum ex*h | sum ex] into a per-window PSUM tile; the window epilogue
    normalizes, averages heads, adds bias, computes prior = kg @ W_prior
    (host-transposed kgT, single preload) and gates the two; outputs are
    staged and written 8 windows per DMA.
All float math on device; host does integer edge partitioning/packing and
dtype/layout conversion of inputs only.
"""
import sys
import numpy as np

if "/opt/trn_rl_repo" not in sys.path:
    sys.path.insert(0, "/opt/trn_rl_repo")

from contextlib import ExitStack

import ml_dtypes

import concourse.bass as bass
import concourse.bacc as bacc
import concourse.mybir as mybir
import concourse.tile as tile
from concourse.bass_utils import run_bass_kernel_spmd

BF16 = ml_dtypes.bfloat16

P = 128
N = 50000
DIN = 128
HEADS = 4
F = 64
HF = HEADS * F          # 256
NCORES = 8
SH = N // NCORES        # 6250 dst nodes per core
ROWE = 384              # T row stride in bf16 elems (768 B)
ROWD = 260              # written part of a T row (520 B): h(256) + asrc(4)
BUCK_N = 25000          # real nodes per source bucket
BROWS = BUCK_N + 1      # bucket rows incl sentinel
SENT = BUCK_N           # bucket-local sentinel index
NB = (BUCK_N + P - 1) // P      # 196 phase-1 blocks per bucket
NBF = BUCK_N // P               # 195 full blocks
WIN = 96                # dst window size (<=128; sized so one (w,b) group
                        # usually fits a single 1024-index dma_gather)
NWIN = (SH + WIN - 1) // WIN    # 66 windows per core
AROWS = 6272            # A-table rows; trash row = SH
KGTC = NWIN * WIN + P   # kgt padded cols (window slices are 128 wide)
ATRASH = SH
NXPAD = 2 * NB * P      # padded xT columns (50176)
ACH = 32                # blocks of a_dst staged per scatter
TB = 15                 # full blocks of T rows staged per write
XB = 8                  # blocks of x loaded per DMA
OB = 8                  # windows of output staged per write
NEG_SLOPE = 0.2
dt = mybir.dt
Alu = mybir.AluOpType
Act = mybir.ActivationFunctionType


# ----------------------------------------------------------------------------
# host-side edge preparation (integer/layout work only)
# ----------------------------------------------------------------------------

def _pack_idx16(idx):
    """Pack int array -> [128, ceil(n/16)] int16 (wrap 16, replicate x8)."""
    n = len(idx)
    cols = (n + 15) // 16
    a = np.zeros((16, cols), np.int16)
    a[np.arange(n) % 16, np.arange(n) // 16] = idx.astype(np.int16)
    return np.tile(a, (8, 1))


def _prep_edges(edge_index):
    src = np.asarray(edge_index[0], dtype=np.int64)
    dst = np.asarray(edge_index[1], dtype=np.int64)
    loops = np.arange(N, dtype=np.int64)
    src = np.concatenate([src, loops])
    dst = np.concatenate([dst, loops])

    core = dst // SH
    dl = dst % SH
    w = dl // WIN
    b = (src >= BUCK_N).astype(np.int64)

    gid = (core * NWIN + w) * 2 + b
    ngroups = NCORES * NWIN * 2
    counts = np.bincount(gid, minlength=ngroups).reshape(NCORES, NWIN, 2)

    # uniform counts across cores (shared NEFF), 16-granular
    n_wb = counts.max(axis=0)                        # [NWIN, 2]
    n16 = np.maximum(((n_wb + 15) // 16) * 16, 16)   # [NWIN, 2]
    g_wb = (n16 + P - 1) // P                        # blocks per group
    gmax = int(g_wb.max())
    s16 = int(n16.sum())                             # total gathered slots
    cblk = int(g_wb.sum())                           # total blocks

    # chunk list: (w, g, blk0, ic0, nidx, tbase, first, last) in emission
    # order; groups larger than GSPL blocks are split so one dma_gather never
    # exceeds GSPL*128 indices (SWDGE descriptor ring capacity)
    GSPL = 8
    chunks = []
    ic = 0
    blk0 = 0
    for wi in range(NWIN):
        for bi in range(2):
            rem = int(g_wb[wi, bi])
            nrem = int(n16[wi, bi])
            while rem > 0:
                gg = min(rem, GSPL)
                nn = min(nrem, gg * P)
                first = bi == 0 and rem == int(g_wb[wi, 0])
                last = bi == 1 and rem == gg
                chunks.append((wi, gg, blk0, ic // 16, nn,
                               bi * BROWS, first, last))
                ic += nn
                blk0 += gg
                rem -= gg
                nrem -= nn
    gmax = min(gmax, GSPL)

    order = np.argsort(gid, kind="stable")
    src_s, dst_s = src[order], dst[order]
    grp_starts = np.zeros(ngroups + 1, np.int64)
    np.cumsum(np.bincount(gid[order], minlength=ngroups), out=grp_starts[1:])

    s_off = np.zeros((NWIN, 2), np.int64)
    b_off = np.zeros((NWIN, 2), np.int64)
    o = 0
    ob = 0
    for wi in range(NWIN):
        for bi in range(2):
            s_off[wi, bi] = o
            b_off[wi, bi] = ob
            o += int(n16[wi, bi])
            ob += int(g_wb[wi, bi])

    midx = np.full((NCORES, s16), SENT, np.int64)    # bucket-local src row
    aidx = np.full((NCORES, s16), ATRASH, np.int64)  # core-local dst row
    dstf = np.full((NCORES, cblk, P), -1.0, np.float32)  # dst local to window

    for c in range(NCORES):
        for wi in range(NWIN):
            for bi in range(2):
                gsl = (c * NWIN + wi) * 2 + bi
                a0, a1 = grp_starts[gsl], grp_starts[gsl + 1]
                n = int(a1 - a0)
                if n == 0:
                    continue
                o = int(s_off[wi, bi])
                es, ed = src_s[a0:a1], dst_s[a0:a1]
                edl = ed % SH
                midx[c, o:o + n] = es - bi * BUCK_N
                aidx[c, o:o + n] = edl
                sl = np.arange(n)
                blk = int(b_off[wi, bi]) + sl // P
                dstf[c, blk, sl % P] = (edl - wi * WIN).astype(np.float32)

    midx_p = np.stack([_pack_idx16(midx[c]) for c in range(NCORES)])
    aidx_p = np.stack([_pack_idx16(aidx[c]) for c in range(NCORES)])
    # dstf2: each value duplicated along a new inner dim of 2 (packed last
    # dim for the DVE 2x one-hot compare): [P, cblk*2]
    dstf2 = np.repeat(dstf.transpose(0, 2, 1), 2, axis=2).astype(BF16)

    # phase-1 a_dst scatter indices: slot = (bkt*NB + bi)*128 + p
    sl = np.arange(2 * NB * P)
    bkt = sl // (NB * P)
    off = sl % (NB * P)
    node = bkt * BUCK_N + off
    valid = (off < BUCK_N) & (node < N)
    sidx = np.full((NCORES, 2 * NB * P), ATRASH, np.int64)
    for c in range(NCORES):
        owned = valid & (node // SH == c)
        sidx[c] = np.where(owned, node - c * SH, ATRASH)
    sidx_p = np.stack([_pack_idx16(sidx[c]) for c in range(NCORES)])

    return chunks, gmax, cblk, s16, midx_p, aidx_p, dstf2, sidx_p


# ----------------------------------------------------------------------------
# device program
# ----------------------------------------------------------------------------

def _build_nc(chunks, gmax, cblk, s16):
    nc = bacc.Bacc("TRN2", target_bir_lowering=False, debug=False,
                   dynamic_dma_scratch_size=16384)

    t_xt = nc.dram_tensor("xt", [DIN, NXPAD], dt.bfloat16, kind="ExternalInput")
    t_wg = nc.dram_tensor("wg", [DIN, HF], dt.float32, kind="ExternalInput")
    t_asb = nc.dram_tensor("attsrc_b", [P, HF], dt.float32, kind="ExternalInput")
    t_adb = nc.dram_tensor("attdst_b", [P, HF], dt.float32, kind="ExternalInput")
    t_bg = nc.dram_tensor("biasg_b", [P, F], dt.float32, kind="ExternalInput")
    t_bp = nc.dram_tensor("bprior_b", [P, F], dt.float32, kind="ExternalInput")
    t_gate = nc.dram_tensor("gate_b", [P, 1], dt.float32, kind="ExternalInput")
    t_wp = nc.dram_tensor("wp16", [32, F], dt.bfloat16, kind="ExternalInput")
    t_kgt = nc.dram_tensor("kgt", [32, KGTC], dt.bfloat16, kind="ExternalInput")
    t_iota = nc.dram_tensor("iota_g", [P, gmax * P], dt.bfloat16,
                            kind="ExternalInput")
    t_midx = nc.dram_tensor("midx", [P, s16 // 16], dt.int16,
                            kind="ExternalInput")
    t_aidx = nc.dram_tensor("aidx", [P, s16 // 16], dt.int16,
                            kind="ExternalInput")
    t_dstf = nc.dram_tensor("dstf2", [P, cblk * 2], dt.bfloat16,
                            kind="ExternalInput")
    t_sidx = nc.dram_tensor("sidx", [P, (2 * NB * P) // 16], dt.int16,
                            kind="ExternalInput")
    t_out = nc.dram_tensor("out", [SH, F], dt.float32, kind="ExternalOutput")

    t_T = nc.dram_tensor("tbl", [2 * BROWS, ROWE], dt.bfloat16)
    t_A = nc.dram_tensor("adst", [AROWS, F], dt.float32)

    def v3(t, d1, s1, d2, s2):
        return bass.AP(t.tensor, t.offset, [t.ap[0], [s1, d1], [s2, d2]])

    def v4(t, d1, s1, d2, s2, d3, s3, off=0):
        return bass.AP(t.tensor, t.offset + off,
                       [t.ap[0], [s1, d1], [s2, d2], [s3, d3]])

    with tile.TileContext(nc) as tc:
        with ExitStack() as octx:
            cp = octx.enter_context(tc.tile_pool(name="const", bufs=1))

            iota_sb = cp.tile([P, gmax * P], dt.bfloat16)
            nc.sync.dma_start(iota_sb[:], t_iota[:])
            wp_sb = cp.tile([32, F], dt.bfloat16)
            nc.sync.dma_start(wp_sb[:], t_wp[:])
            kgt_sb = cp.tile([32, KGTC], dt.bfloat16)
            nc.sync.dma_start(kgt_sb[:], t_kgt[:])
            asb = cp.tile([P, HF], dt.float32)
            nc.sync.dma_start(asb[:], t_asb[:])
            adb = cp.tile([P, HF], dt.float32)
            nc.sync.dma_start(adb[:], t_adb[:])
            bg_sb = cp.tile([P, F], dt.float32)
            nc.sync.dma_start(bg_sb[:], t_bg[:])
            bp_sb = cp.tile([P, F], dt.float32)
            nc.sync.dma_start(bp_sb[:], t_bp[:])
            gate_sb = cp.tile([P, 1], dt.float32)
            nc.sync.dma_start(gate_sb[:], t_gate[:])
            g1m_sb = cp.tile([P, 1], dt.float32)
            nc.vector.tensor_scalar(g1m_sb[:], gate_sb[:], -1.0, 1.0,
                                    op0=Alu.mult, op1=Alu.add)
            cb_sb = cp.tile([P, F], dt.float32)
            nc.vector.tensor_scalar_mul(cb_sb[:], bg_sb[:], g1m_sb[:, 0:1])
            cb2 = cp.tile([P, F], dt.float32)
            nc.vector.tensor_scalar_mul(cb2[:], bp_sb[:], gate_sb[:, 0:1])
            nc.vector.tensor_tensor(out=cb_sb[:], in0=cb_sb[:], in1=cb2[:],
                                    op=Alu.add)
            midx_sb = cp.tile([P, s16 // 16], dt.int16)
            nc.sync.dma_start(midx_sb[:], t_midx[:])
            aidx_sb = cp.tile([P, s16 // 16], dt.int16)
            nc.sync.dma_start(aidx_sb[:], t_aidx[:])
            dstf_sb = cp.tile([P, cblk * 2], dt.bfloat16)
            nc.sync.dma_start(dstf_sb[:], t_dstf[:])
            sidx_sb = cp.tile([P, (2 * NB * P) // 16], dt.int16)
            nc.sync.dma_start(sidx_sb[:], t_sidx[:])

            # W_ext = [W_gat | wsrc(4) | wdst(4)] in (j, head) interleave;
            # wsrc_hd = sum_j W[:,(j,hd)]*att_src[(j,hd)]
            wext = cp.tile([P, HF + 8], dt.float32)
            nc.sync.dma_start(wext[:, 0:HF], t_wg[:])
            with tc.tile_pool(name="wtmp", bufs=2) as wtp:
                for j in range(HEADS):
                    for k, att in enumerate((asb, adb)):
                        tmp = wtp.tile([P, F], dt.float32, tag="wtmp")
                        nc.vector.tensor_tensor(
                            out=tmp[:],
                            in0=v3(bass.AP(wext[:].tensor, wext[:].offset + j,
                                           [wext[:].ap[0]]), F, 4, 1, 1),
                            in1=v3(bass.AP(att[:].tensor, att[:].offset + j,
                                           [att[:].ap[0]]), F, 4, 1, 1),
                            op=Alu.mult)
                        nc.vector.tensor_reduce(
                            out=wext[:, HF + 4 * k + j:HF + 4 * k + j + 1],
                            in_=tmp[:], axis=mybir.AxisListType.X, op=Alu.add)
            wext16 = cp.tile([P, HF + 8], dt.bfloat16)
            nc.vector.tensor_copy(out=wext16[:], in_=wext[:])

            # zero-fill the A-table (scatter_add target)
            zt = cp.tile([P, (AROWS // P) * F], dt.float32)
            nc.vector.memset(zt[:], 0.0)
            nc.sync.dma_start(
                t_A[:, :].rearrange("(p a) c -> p (a c)", p=P), zt[:])

            # ---------------- phase 1: build T and A ----------------
            with tc.tile_pool(name="p1x", bufs=3) as p1x, \
                 tc.tile_pool(name="p1r", bufs=3) as p1r, \
                 tc.tile_pool(name="p1s", bufs=2) as p1s, \
                 tc.tile_pool(name="p1ps", bufs=4, space="PSUM") as pp1, \
                 tc.tile_pool(name="p1ac", bufs=3) as p1a:
                xb = None
                rb = None
                ach_tile = None
                for bkt in range(2):
                    for bi in range(NB):
                        if bi % XB == 0:
                            colb = bkt * BUCK_N + bi * P
                            nx = min(XB, NB - bi) * P
                            xb = p1x.tile([P, XB * P], dt.bfloat16, tag="xb")
                            nc.sync.dma_start(xb[:, 0:nx],
                                              t_xt[:, colb:colb + nx])
                        h_ps = pp1.tile([P, HF + 8], dt.float32, space="PSUM",
                                        tag="hps")
                        nc.tensor.matmul(
                            h_ps[:],
                            lhsT=xb[:, (bi % XB) * P:(bi % XB + 1) * P],
                            rhs=wext16[:], start=True, stop=True)
                        # T-row staging (TB full blocks per write)
                        full = bi < NBF
                        if full:
                            ti = (bi % TB)
                            if ti == 0:
                                rb = p1r.tile([P, TB, ROWD], dt.bfloat16,
                                              tag="rb")
                            nc.vector.tensor_copy(out=rb[:, ti, 0:96],
                                                  in_=h_ps[:, 0:96])
                            nc.scalar.copy(rb[:, ti, 96:ROWD],
                                           h_ps[:, 96:ROWD])
                            if ti == TB - 1 or bi == NBF - 1:
                                nb = ti + 1
                                r0 = bkt * BROWS + (bi - ti) * P
                                dst_ap = t_T[r0:r0 + nb * P, 0:ROWD] \
                                    .rearrange("(b p) c -> p b c", p=P)
                                nc.sync.dma_start(dst_ap, rb[:, 0:nb, :])
                        else:
                            # partial tail block: write its rows directly
                            nr = BUCK_N - bi * P
                            rt = p1s.tile([P, ROWD], dt.bfloat16, tag="rt")
                            nc.vector.tensor_copy(out=rt[:, 0:96],
                                                  in_=h_ps[:, 0:96])
                            nc.scalar.copy(rt[:, 96:ROWD], h_ps[:, 96:ROWD])
                            nc.sync.dma_start(
                                t_T[bkt * BROWS + bi * P:
                                    bkt * BROWS + bi * P + nr, 0:ROWD],
                                rt[:nr, :])
                        ci = bi % ACH
                        if ci == 0:
                            ach_tile = p1a.tile([P, ACH, HEADS], dt.float32,
                                                tag="ach")
                        nc.vector.tensor_copy(out=ach_tile[:, ci, :],
                                              in_=h_ps[:, HF + 4:HF + 8])
                        if ci == ACH - 1 or bi == NB - 1:
                            nb = ci + 1
                            slot0 = (bkt * NB + bi - ci) * P
                            nc.gpsimd.dma_scatter_add(
                                t_A[:, 0:HEADS], ach_tile[:, 0:nb, :],
                                sidx_sb[:, slot0 // 16:(slot0 + nb * P) // 16],
                                nb * P, nb * P, HEADS, F)
                # sentinel rows (h=0, a_src=-1e30)
                sent = p1s.tile([1, ROWD], dt.bfloat16, tag="sent")
                nc.gpsimd.memset(sent[:], 0.0)
                nc.gpsimd.memset(sent[:, HF:ROWD], -1e30)
                nc.sync.dma_start(t_T[SENT:SENT + 1, 0:ROWD], sent[:])
                nc.sync.dma_start(t_T[BROWS + SENT:BROWS + SENT + 1, 0:ROWD],
                                  sent[:])

            # ---------------- phase 2: gather / scatter ----------------
            with tc.tile_pool(name="g1p", bufs=4) as g1p, \
                 tc.tile_pool(name="g2p", bufs=4) as g2p, \
                 tc.tile_pool(name="wk", bufs=4) as wk, \
                 tc.tile_pool(name="sp", bufs=4) as sp, \
                 tc.tile_pool(name="accp", bufs=4, space="PSUM") as accp, \
                 tc.tile_pool(name="prp", bufs=2, space="PSUM") as prp, \
                 tc.tile_pool(name="fin", bufs=4) as fin, \
                 tc.tile_pool(name="outp", bufs=2) as outp:

                # one-time init: tail slots of partially-gathered blocks
                # are read (and zeroed via S) but must hold finite data
                for _ in range(4):
                    gw1 = g1p.tile([P, gmax, ROWE], dt.bfloat16, tag="g1")
                    nc.vector.memset(gw1[:], 0.0)
                    gw2 = g2p.tile([P, gmax, F], dt.float32, tag="g2")
                    nc.vector.memset(gw2[:], 0.0)

                acc_ps = None
                ob_tile = None
                for (wi, g, blk0, ic0, nn, tbase, first, last) in chunks:
                    if first:
                        acc_ps = accp.tile([P, HF + 4], dt.float32,
                                           space="PSUM", tag="acc")
                    g1 = g1p.tile([P, gmax, ROWE], dt.bfloat16, tag="g1")
                    nc.gpsimd.dma_gather(
                        g1[:, 0:g, :], t_T[tbase:tbase + BROWS, :],
                        midx_sb[:, ic0:ic0 + nn // 16], nn, nn, ROWE)
                    g2 = g2p.tile([P, gmax, F], dt.float32, tag="g2")
                    nc.gpsimd.dma_gather(
                        g2[:, 0:g, :], t_A[:, :],
                        aidx_sb[:, ic0:ic0 + nn // 16], nn, nn, F)

                    g1a = g1[:, 0:g, :]
                    # alpha = leaky(asrc + adst)
                    alpha = wk.tile([P, gmax * HEADS], dt.float32, tag="alpha")
                    nc.vector.tensor_tensor(
                        out=v3(alpha[:], g, 4, 4, 1),
                        in0=bass.AP(g1a.tensor, g1a.offset + HF,
                                    [g1a.ap[0], [ROWE, g], [1, 4]]),
                        in1=bass.AP(g2[:].tensor, g2[:].offset,
                                    [g2[:].ap[0], [F, g], [1, 4]]),
                        op=Alu.add)
                    nc.vector.scalar_tensor_tensor(
                        out=alpha[:, 0:g * 4], in0=alpha[:, 0:g * 4],
                        scalar=NEG_SLOPE, in1=alpha[:, 0:g * 4],
                        op0=Alu.mult, op1=Alu.max)
                    ex = wk.tile([P, gmax * HEADS], dt.bfloat16, tag="ex")
                    nc.scalar.activation(ex[:, 0:g * 4], alpha[:, 0:g * 4],
                                         Act.Exp)
                    # S[e, d] = (dstf == d)  [P, g, 128] bf16, 2x via dup'd
                    # dstf and (d1,d0) split so every last dim is packed
                    S = sp.tile([P, gmax * P], dt.bfloat16, tag="S")
                    nc.vector.tensor_tensor(
                        out=v4(S[:], g, P, F, 2, 2, 1),
                        in0=v4(dstf_sb[:, blk0 * 2:(blk0 + g) * 2],
                               g, 2, F, 0, 2, 1),
                        in1=v4(iota_sb[:, 0:g * P], g, 0, F, 2, 2, 1),
                        op=Alu.is_equal)
                    # rhs = [ex*h (256, (j,hd) order) | ex (4)]  [P, g, 260]
                    rhs = wk.tile([P, gmax * (HF + 4)], dt.bfloat16, tag="rhs")
                    nc.vector.tensor_tensor(
                        out=v4(rhs[:], g, HF + 4, F, 4, 4, 1),
                        in0=v4(g1a, g, ROWE, F, 4, 4, 1),
                        in1=v4(ex[:], g, 4, F, 0, 4, 1),
                        op=Alu.mult)
                    nc.scalar.copy(
                        bass.AP(rhs[:].tensor, rhs[:].offset + HF,
                                [rhs[:].ap[0], [HF + 4, g], [1, 4]]),
                        v3(ex[:], g, 4, 4, 1))
                    for k in range(g):
                        nc.tensor.matmul(
                            acc_ps[:],
                            lhsT=S[:, k * P:(k + 1) * P],
                            rhs=rhs[:, k * (HF + 4):(k + 1) * (HF + 4)],
                            start=(first and k == 0),
                            stop=(last and k == g - 1))

                    if last:
                        # ---- window epilogue (acc cols are (j, hd)) ----
                        nw = min(WIN, SH - wi * WIN)
                        den = fin.tile([P, HEADS], dt.float32, tag="den")
                        nc.vector.tensor_scalar(
                            den[:], acc_ps[:, HF:HF + 4], 1e-16, float(HEADS),
                            op0=Alu.add, op1=Alu.mult)
                        rec = fin.tile([P, HEADS], dt.float32, tag="rec")
                        nc.vector.reciprocal(rec[:], den[:])
                        prod = fin.tile([P, HF], dt.float32, tag="prod")
                        nc.vector.tensor_tensor(
                            out=v4(prod[:], F, 4, 1, 0, 4, 1),
                            in0=v4(acc_ps[:, 0:HF], F, 4, 1, 0, 4, 1),
                            in1=v4(rec[:], F, 0, 1, 0, 4, 1), op=Alu.mult)
                        oi = wi % OB
                        if oi == 0:
                            ob_tile = outp.tile([P, OB, F], dt.float32,
                                                tag="ot")
                        gat = fin.tile([P, F], dt.float32, tag="gat")
                        nc.vector.tensor_reduce(
                            out=gat[:],
                            in_=v3(prod[:], F, 4, HEADS, 1),
                            axis=mybir.AxisListType.X, op=Alu.add)
                        # prior = kgT_win.T @ W_prior
                        pr_ps = prp.tile([P, F], dt.float32, space="PSUM",
                                         tag="prps")
                        nc.tensor.matmul(pr_ps[:],
                                         lhsT=kgt_sb[:, wi * WIN:wi * WIN + P],
                                         rhs=wp_sb[:], start=True, stop=True)
                        # combine: (1-g)*(gat+bias) + g*(prior+bprior)
                        t1 = ob_tile[:, oi, :]
                        nc.vector.tensor_scalar_mul(t1, gat[:], g1m_sb[:, 0:1])
                        t2 = fin.tile([P, F], dt.float32, tag="t2")
                        nc.vector.tensor_scalar_mul(t2[:], pr_ps[:],
                                                    gate_sb[:, 0:1])
                        nc.vector.tensor_tensor(out=t1, in0=t1, in1=t2[:],
                                                op=Alu.add)
                        nc.vector.tensor_tensor(out=t1, in0=t1, in1=cb_sb[:],
                                                op=Alu.add)
                        if oi == OB - 1 or wi == NWIN - 1:
                            nbw = oi + 1
                            w0 = wi - oi
                            nrows = min(OB * WIN, SH - w0 * WIN)
                            if nrows == nbw * WIN:
                                dst_ap = t_out[w0 * WIN:w0 * WIN + nrows, :] \
                                    .rearrange("(b p) c -> p b c", p=WIN)
                                nc.sync.dma_start(dst_ap,
                                                  ob_tile[:WIN, 0:nbw, :])
                            else:
                                # tail: full windows then the partial one
                                if nbw > 1:
                                    da = t_out[w0 * WIN:
                                               w0 * WIN + (nbw - 1) * WIN,
                                               :].rearrange(
                                        "(b p) c -> p b c", p=WIN)
                                    nc.sync.dma_start(
                                        da, ob_tile[:WIN, 0:nbw - 1, :])
                                nc.sync.dma_start(
                                    t_out[wi * WIN:wi * WIN + nw, :],
                                    ob_tile[:nw, nbw - 1, :])

    nc.compile()
    return nc


def _interleave_cols(a):
    """[..., HEADS*F] head-major -> (j, head) interleaved columns."""
    s = a.shape[:-1]
    return np.ascontiguousarray(
        a.reshape(s + (HEADS, F)).swapaxes(-1, -2).reshape(s + (HEADS * F,)))


def kernel(**inputs):
    x = np.asarray(inputs["x"], np.float32)
    edge_index = np.asarray(inputs["edge_index"])
    kg = np.asarray(inputs["kg_onehot"], np.float32)
    wg = np.ascontiguousarray(np.asarray(inputs["W_gat"], np.float32))
    att_src = np.asarray(inputs["att_src"], np.float32)
    att_dst = np.asarray(inputs["att_dst"], np.float32)
    bias_gat = np.asarray(inputs["bias_gat"], np.float32)
    wp = np.asarray(inputs["W_prior"], np.float32)
    b_prior = np.asarray(inputs["b_prior"], np.float32)
    gate = np.asarray(inputs["gate"], np.float32)

    chunks, gmax, cblk, s16, midx_p, aidx_p, dstf2, sidx_p = \
        _prep_edges(edge_index)

    xt = np.zeros((DIN, NXPAD), BF16)
    xt[:, 0:N] = np.ascontiguousarray(x.T).astype(BF16)
    iota_g = np.broadcast_to(
        np.tile(np.arange(P, dtype=np.float32), gmax)[None, :],
        (P, gmax * P)).astype(BF16)

    wg_i = _interleave_cols(wg)                      # [128, 256] (j, hd)
    att_src_i = _interleave_cols(att_src.reshape(1, HF))
    att_dst_i = _interleave_cols(att_dst.reshape(1, HF))

    shared = {
        "xt": xt, "wg": wg_i,
        "wp16": wp.astype(BF16),
        "attsrc_b": np.broadcast_to(att_src_i, (P, HF)).copy(),
        "attdst_b": np.broadcast_to(att_dst_i, (P, HF)).copy(),
        "biasg_b": np.broadcast_to(bias_gat.reshape(1, F), (P, F)).copy(),
        "bprior_b": np.broadcast_to(b_prior.reshape(1, F), (P, F)).copy(),
        "gate_b": np.broadcast_to(gate.reshape(1, 1), (P, 1)).copy(),
        "iota_g": np.ascontiguousarray(iota_g),
    }
    in_maps = []
    for c in range(NCORES):
        m = dict(shared)
        m["midx"] = midx_p[c]
        m["aidx"] = aidx_p[c]
        m["dstf2"] = dstf2[c]
        m["sidx"] = sidx_p[c]
        kgt = np.zeros((32, KGTC), BF16)
        kgt[:, 0:SH] = kg[c * SH:(c + 1) * SH].T.astype(BF16)
        m["kgt"] = kgt
        in_maps.append(m)

    nc = _build_nc(chunks, gmax, cblk, s16)
    res = run_bass_kernel_spmd(nc, in_maps, core_ids=list(range(NCORES)))
    out = np.concatenate([res.results[c]["out"] for c in range(NCORES)],
                         axis=0)
    return out.astype(np.float32)
